# revision 1
# baseline (speedup 1.0000x reference)
"""Trainium2 Bass kernel for nn_LocalTransformer (4-layer transformer,
d=1024, 16 heads, dff=4096, seq=1024, batch=4, causal + 64-lookahead mask).

Sharding: 8 cores = 4 samples x 2 sequence halves; each core owns 512
tokens. Attention context is a relative window of 1152 positions
(p = t - qoff + 512) so the mask predicate `p <= q + 576` is
core-independent; out-of-window positions are killed by per-core pad
biases added inside exp and an affine_select on the boundary chunks.
K/V for remote positions move via a pairwise AllGather per layer; the
receiver computes peer = slot0 + slot1 - own (exact in fp32 for bf16).

Precision: projections/FFN in fp32r (TF32-like, full PE rate at N=512),
attention inner products in bf16, activations/residual fp32.
"""
import numpy as np

L, D, H, DFF, S, B = 4, 1024, 16, 4096, 1024, 4
HD = D // H  # 64
T = 512  # local tokens per core
WIN = 1152  # kv window positions (9 chunks of 128)
NC = 9
EPS = 1e-5
NEG = -30000.0
V_E = H * 65  # 1040: per head [V(64) | denominator-ones col]

_CACHE = {}


def _build_program():
    import concourse.bass as bass
    import concourse.tile as tile
    from concourse import bacc, mybir
    from contextlib import ExitStack

    f32, bf16, f32r = mybir.dt.float32, mybir.dt.bfloat16, mybir.dt.float32r
    AF = mybir.ActivationFunctionType
    ALU = mybir.AluOpType

    nc = bacc.Bacc("TRN2", target_bir_lowering=False, debug=False, num_devices=8)

    I = {}
    I["x0"] = nc.dram_tensor("x0", [D, T], f32r, kind="ExternalInput").ap()
    I["pb"] = nc.dram_tensor("pb", [NC, 128, 1], f32, kind="ExternalInput").ap()
    for k, sh in (
        ("wqT", [L, D, D]),
        ("wkT", [L, D, D]),
        ("wvT", [L, D + 1, V_E]),
        ("woT", [L, D, D]),
        ("w1T", [L, D, DFF]),
        ("w2T", [L, DFF, D]),
        ("ones1", [1, 128]),
        ("onesd", [128, 1]),
        ("onesr", [1, 512]),

        ("bq", [L, 8, 128, 1]),
        ("bk", [L, 8, 128, 1]),
        ("bo", [L, 8, 128, 1]),
        ("b1", [L, 32, 128, 1]),
        ("b2", [L, 8, 128, 1]),
        ("ln1g", [L, 8, 128, 1]),
        ("ln1b", [L, 8, 128, 1]),
        ("ln2g", [L, 8, 128, 1]),
        ("ln2b", [L, 8, 128, 1]),
    ):
        I[k] = nc.dram_tensor(
            k, sh, f32r if k[0] in "wo" and k != "bo" else f32, kind="ExternalInput"
        ).ap()
    y = nc.dram_tensor("y", [D, T], f32, kind="ExternalOutput").ap()

    cck_in, cck_out, ccv_in, ccv_out = [], [], [], []
    for l in range(L):
        cck_in.append(nc.dram_tensor(f"ccki{l}", [D, 576], bf16, kind="Internal").ap())
        cck_out.append(nc.dram_tensor(f"ccko{l}", [2 * D, 576], bf16, kind="Internal").ap())
        ccv_in.append(nc.dram_tensor(f"ccvi{l}", [576, V_E], bf16, kind="Internal").ap())
        ccv_out.append(nc.dram_tensor(f"ccvo{l}", [1152, V_E], bf16, kind="Internal").ap())

    RG = [[0, 1], [2, 3], [4, 5], [6, 7]]
    SELEXT = {4: 64, 5: 192, 6: 320, 7: 448, 8: 512}
    HGROUPS = [range(0, 6), range(6, 12), range(12, 16)]

    with tile.TileContext(nc) as tc, ExitStack() as ctx:
        pers = ctx.enter_context(tc.tile_pool(name="pers", bufs=1))
        X = [pers.tile([128, T], f32r, tag=f"X{i}", name=f"X{i}") for i in range(8)]
        X2 = [pers.tile([128, T], f32r, tag=f"X2{i}", name=f"X2{i}") for i in range(8)]
        OP = [pers.tile([128, T], f32r, tag=f"OP{i}", name=f"OP{i}") for i in range(8)]
        Q = [pers.tile([128, T], bf16, tag=f"Q{i}", name=f"Qt{i}") for i in range(8)]
        KH = [pers.tile([128, WIN], bf16, tag=f"KH{i}", name=f"KHt{i}") for i in range(8)]
        VT = [pers.tile([128, V_E], bf16, tag=f"VT{i}", name=f"VTt{i}") for i in range(NC)]
        ACC = [pers.tile([128, T], f32r, tag=f"ACC{i}", name=f"ACCt{i}") for i in range(8)]
        ones_row = pers.tile([1, T], f32r, tag="ones_row", name="ones_row")
        ones1 = pers.tile([1, 128], f32r, tag="ones1", name="ones1t")
        onesd_t = pers.tile([128, 1], f32r, tag="onesd", name="onesdt")
        pb_t = [pers.tile([128, 1], f32, tag=f"pb{i}", name=f"pbt{i}") for i in range(NC)]
        D16 = pers.tile([16, T], f32, tag="D16", name="D16")
        R16 = pers.tile([16, T], f32, tag="R16", name="R16")

        wp = ctx.enter_context(tc.tile_pool(name="wp", bufs=4))  # [128,1024] panels
        smw = ctx.enter_context(tc.tile_pool(name="smw", bufs=3))  # small weights
        pp = ctx.enter_context(tc.tile_pool(name="pp", bufs=2, space="PSUM"))
        pav = ctx.enter_context(tc.tile_pool(name="pav", bufs=6, space="PSUM"))
        pr = ctx.enter_context(tc.tile_pool(name="pr", bufs=5))  # probs bf16
        tpv = ctx.enter_context(tc.tile_pool(name="tpv", bufs=2))  # [128,1040] f32
        tps = ctx.enter_context(tc.tile_pool(name="tps", bufs=3))  # [128,T] f32
        tpr = ctx.enter_context(tc.tile_pool(name="tpr", bufs=2))  # [128,T] f32r
        fx = ctx.enter_context(tc.tile_pool(name="fx", bufs=2))  # fixup bf16 loads
        sm = ctx.enter_context(tc.tile_pool(name="sm", bufs=3))  # [1,T] smalls
        fbp = ctx.enter_context(tc.tile_pool(name="fbp", bufs=8))  # FFN blocks
        rhp = ctx.enter_context(tc.tile_pool(name="rhp", bufs=2))  # [1,T] recip
        lp = ctx.enter_context(tc.tile_pool(name="lp", bufs=4))  # [128,1] params

        r = lambda ap: ap.bitcast(f32r)
        q32 = lambda ap: ap.bitcast(f32)

        nc.sync.dma_start(out=ones_row[:], in_=I["onesr"][:])
        nc.sync.dma_start(out=ones1[:], in_=I["ones1"][:])
        nc.sync.dma_start(out=onesd_t[:], in_=I["onesd"][:])
        for i in range(NC):
            nc.sync.dma_start(out=pb_t[i][:], in_=I["pb"][i])
        for i in range(8):
            nc.sync.dma_start(out=X[i][:], in_=I["x0"][i * 128 : (i + 1) * 128, :])
        for i in range(8):
            nc.gpsimd.memset(KH[i][:], 0.0)
        for i in range(NC):
            nc.gpsimd.memset(VT[i][:], 0.0)

        def wpanel(src2d, cols):
            """DMA a [1024, 128] column-slice as [128, 8, 128] panel."""
            pan = wp.tile([128, 8, 128], f32r, tag="wpan", name="wpan")
            nc.sync.dma_start(
                out=pan[:],
                in_=src2d[:, cols].rearrange("(dc p) c -> p dc c", p=128),
            )
            return pan

        def ln_param(src, l, dc):
            t = lp.tile([128, 1], f32, tag="lnp", name="lnp")
            nc.sync.dma_start(out=t[:], in_=src[l, dc])
            return t

        for l in range(L):
            # ---------- K projection -> KH[:, 512:1024], stage, AllGather ----------
            for ec in range(8):
                pan = wpanel(I["wkT"][l], slice(ec * 128, (ec + 1) * 128))
                ps = pp.tile([128, T], f32, tag="ps", name="ps")
                for dc in range(8):
                    nc.tensor.matmul(
                        out=ps[:], lhsT=r(pan[:, dc, :]), rhs=r(X[dc][:]),
                        start=(dc == 0), stop=(dc == 7),
                    )
                bt = ln_param(I["bk"], l, ec)
                nc.scalar.activation(
                    KH[ec][:, 512:1024], ps[:], AF.Identity, bias=bt[:], scale=1.0
                )
                nc.sync.dma_start(
                    out=cck_in[l][ec * 128 : (ec + 1) * 128, 0:512],
                    in_=KH[ec][:, 512:1024],
                )
                nc.sync.dma_start(
                    out=cck_in[l][ec * 128 : (ec + 1) * 128, 512:576],
                    in_=KH[ec][:, 512:576],
                )
            nc.gpsimd.collective_compute(
                "AllGather", mybir.AluOpType.bypass, replica_groups=RG,
                ins=[cck_in[l][:]], outs=[cck_out[l][:]],
            )

            # ---------- V projection -> VT[4..7], stage, AllGather ----------
            for tcx in range(4):
                for s0, s1 in ((0, 512), (512, 1024), (1024, V_E)):
                    w = s1 - s0
                    ps = pp.tile([128, T], f32, tag="ps", name="ps")
                    for dc in range(8):
                        pan = smw.tile([128, 512], f32r, tag="vpan", name="vpan")
                        nc.sync.dma_start(
                            out=pan[:, 0:w],
                            in_=I["wvT"][l, dc * 128 : (dc + 1) * 128, s0:s1],
                        )
                        nc.tensor.matmul(
                            out=ps[:, 0:w],
                            lhsT=r(X[dc][:, tcx * 128 : (tcx + 1) * 128]),
                            rhs=r(pan[:, 0:w]),
                            start=(dc == 0), stop=False,
                        )
                    bia = smw.tile([1, 512], f32r, tag="vbias", name="vbias")
                    nc.sync.dma_start(out=bia[:, 0:w], in_=I["wvT"][l, D : D + 1, s0:s1])
                    nc.tensor.matmul(
                        out=ps[:, 0:w],
                        lhsT=r(ones_row[:, tcx * 128 : (tcx + 1) * 128]),
                        rhs=r(bia[:, 0:w]),
                        start=False, stop=True,
                    )
                    nc.scalar.activation(VT[4 + tcx][:, s0:s1], ps[:, 0:w], AF.Copy)
                nc.sync.dma_start(
                    out=ccv_in[l][tcx * 128 : (tcx + 1) * 128, :], in_=VT[4 + tcx][:]
                )
            nc.sync.dma_start(out=ccv_in[l][512:576, :], in_=VT[4][0:64, :])
            nc.gpsimd.collective_compute(
                "AllGather", mybir.AluOpType.bypass, replica_groups=RG,
                ins=[ccv_in[l][:]], outs=[ccv_out[l][:]],
            )
            for pc in range(4, 8):  # local denominator ones-columns
                nc.gpsimd.memset(
                    VT[pc][:].rearrange("p (h c) -> p h c", c=65)[:, :, 64:65], 1.0
                )

            # ---------- Q projection ----------
            for ec in range(8):
                pan = wpanel(I["wqT"][l], slice(ec * 128, (ec + 1) * 128))
                ps = pp.tile([128, T], f32, tag="ps", name="ps")
                for dc in range(8):
                    nc.tensor.matmul(
                        out=ps[:], lhsT=r(pan[:, dc, :]), rhs=r(X[dc][:]),
                        start=(dc == 0), stop=(dc == 7),
                    )
                bt = ln_param(I["bq"], l, ec)
                nc.scalar.activation(Q[ec][:], ps[:], AF.Identity, bias=bt[:], scale=1.0)

            # ---------- remote KV fixup (consumes AllGather results) ----------
            for ec in range(8):
                ta = fx.tile([128, 576], bf16, tag="fxa", name="fxa")
                tb = fx.tile([128, 576], bf16, tag="fxb", name="fxb")
                nc.sync.dma_start(out=ta[:], in_=cck_out[l][ec * 128 : (ec + 1) * 128, :])
                nc.sync.dma_start(
                    out=tb[:], in_=cck_out[l][D + ec * 128 : D + (ec + 1) * 128, :]
                )
                ts = tpv.tile([128, 1040], f32, tag="fxv", name="fxv")
                nc.vector.tensor_add(ts[:, 0:576], ta[:], tb[:])
                nc.vector.tensor_sub(
                    KH[ec][:, 0:512], ts[:, 0:512], KH[ec][:, 512:1024]
                )
                nc.vector.tensor_sub(
                    KH[ec][:, 1024:1088], ts[:, 512:576], KH[ec][:, 512:576]
                )
            for pc in range(4):
                ta = fx.tile([128, V_E], bf16, tag="fxa", name="fxa")
                tb = fx.tile([128, V_E], bf16, tag="fxb", name="fxb")
                nc.sync.dma_start(out=ta[:], in_=ccv_out[l][pc * 128 : (pc + 1) * 128, :])
                nc.sync.dma_start(
                    out=tb[:], in_=ccv_out[l][576 + pc * 128 : 576 + (pc + 1) * 128, :]
                )
                ts = tpv.tile([128, 1040], f32, tag="fxv", name="fxv")
                nc.vector.tensor_add(ts[:], ta[:], tb[:])
                nc.vector.tensor_sub(VT[pc][:], ts[:], VT[4 + pc][:])
            ta = fx.tile([128, V_E], bf16, tag="fxa", name="fxa")
            tb = fx.tile([128, V_E], bf16, tag="fxb", name="fxb")
            nc.sync.dma_start(out=ta[0:64, :], in_=ccv_out[l][512:576, :])
            nc.sync.dma_start(out=tb[0:64, :], in_=ccv_out[l][1088:1152, :])
            ts = tpv.tile([128, 1040], f32, tag="fxv", name="fxv")
            nc.vector.tensor_add(ts[0:64, :], ta[0:64, :], tb[0:64, :])
            nc.vector.tensor_sub(VT[8][0:64, :], ts[0:64, :], VT[4][0:64, :])
            for pc in (0, 1, 2, 3, 8):
                nc.gpsimd.memset(
                    VT[pc][:].rearrange("p (h c) -> p h c", c=65)[:, :, 64:65], 1.0
                )

            # ---------- attention, head groups with batched reciprocal ----------
            avps = {}
            for grp in HGROUPS:
                for h in grp:
                    par, kc = h % 2, h // 2
                    rows = slice(par * 64, par * 64 + 64)
                    av = pav.tile([128, T], f32, tag="av", name="av")
                    avps[h] = av
                    for idx, c in enumerate([4, 5, 6, 7, 0, 1, 2, 3, 8]):
                        sc = pp.tile([128, T], f32, tag="ps", name="ps")
                        nc.tensor.matmul(
                            out=sc[:],
                            lhsT=KH[kc][rows, c * 128 : (c + 1) * 128],
                            rhs=Q[kc][rows, :],
                            start=True, stop=True,
                        )
                        pt = pr.tile([128, T], bf16, tag="probs", name="probs")
                        nc.scalar.activation(
                            pt[:], sc[:], AF.Exp, bias=pb_t[c][:], scale=1.0
                        )
                        if c in SELEXT:
                            ext = SELEXT[c]
                            nc.gpsimd.affine_select(
                                out=pt[:, 0:ext], in_=pt[:, 0:ext],
                                pattern=[[1, ext]], compare_op=ALU.is_ge,
                                fill=0.0, base=576 - c * 128, channel_multiplier=-1,
                            )
                        nc.tensor.matmul(
                            out=av[0:65, :],
                            lhsT=VT[c][:, h * 65 : h * 65 + 65],
                            rhs=pt[:],
                            start=(idx == 0), stop=(idx == 8),
                        )
                    dt_ = tps.tile([128, T], f32, tag="t512", name="t512")
                    nc.vector.tensor_copy(dt_[64:65, :], av[64:65, :])
                    j = h - grp.start
                    nc.sync.dma_start(out=D16[j : j + 1, :], in_=dt_[64:65, :])
                gs = grp.stop - grp.start
                nc.vector.reciprocal(R16[0:gs, :], D16[0:gs, :])
                for h in grp:
                    j = h - grp.start
                    rht = rhp.tile([1, T], f32r, tag="rh", name="rht")
                    nc.sync.dma_start(out=rht[:], in_=R16[j : j + 1, :].bitcast(f32r))
                    bc = pp.tile([128, T], f32, tag="ps", name="ps")
                    nc.tensor.matmul(
                        out=bc[0:64, :], lhsT=r(ones1[:, 0:64]), rhs=r(rht[:]),
                        start=True, stop=True,
                    )
                    bcs = tps.tile([128, T], f32, tag="t512", name="t512")
                    nc.scalar.activation(bcs[0:64, :], bc[0:64, :], AF.Copy)
                    if h % 2 == 0:
                        nc.vector.tensor_mul(
                            OP[h // 2][0:64, :], avps[h][0:64, :], bcs[0:64, :]
                        )
                    else:
                        ot = tpr.tile([128, T], f32r, tag="otr", name="otr")
                        nc.vector.tensor_mul(
                            ot[0:64, :], avps[h][0:64, :], bcs[0:64, :]
                        )
                        nc.sync.dma_start(
                            out=OP[h // 2][64:128, :], in_=ot[0:64, :]
                        )

            def layernorm(src, gsrc, bsrc, dst):
                mu = pp.tile([1, T], f32, tag="ps", name="ps")
                ms = pp.tile([1, T], f32, tag="ps", name="ps")
                for dc in range(8):
                    sq = tpr.tile([128, T], f32r, tag="sqr", name="sqr")
                    nc.scalar.square(sq[:], q32(src[dc][:]))
                    nc.tensor.matmul(
                        out=mu[:], lhsT=r(onesd_t[:]), rhs=r(src[dc][:]),
                        start=(dc == 0), stop=(dc == 7),
                    )
                    nc.tensor.matmul(
                        out=ms[:], lhsT=r(onesd_t[:]), rhs=r(sq[:]),
                        start=(dc == 0), stop=(dc == 7),
                    )
                mu_sb = sm.tile([1, T], f32r, tag="sm1", name="mu")
                nc.vector.tensor_copy(mu_sb[:], mu[:])
                t2 = sm.tile([1, T], f32, tag="sm1", name="t2")
                nc.vector.tensor_mul(t2[:], q32(mu_sb[:]), q32(mu_sb[:]))
                var = sm.tile([1, T], f32, tag="sm1", name="var")
                nc.vector.tensor_sub(var[:], ms[:], t2[:])
                nc.vector.tensor_scalar_add(var[:], var[:], EPS)
                std = sm.tile([1, T], f32, tag="sm1", name="std")
                nc.scalar.sqrt(std[:], var[:])
                rstd = sm.tile([1, T], f32r, tag="sm1", name="rstd")
                with nc.allow_low_precision(reason="f32r is fp32-width storage"):
                    nc.vector.reciprocal(rstd[:], std[:])
                mub = pp.tile([128, T], f32, tag="ps", name="ps")
                nc.tensor.matmul(
                    out=mub[:], lhsT=r(ones1[:]), rhs=r(mu_sb[:]), start=True, stop=True
                )
                rsb = pp.tile([128, T], f32, tag="ps", name="ps")
                nc.tensor.matmul(
                    out=rsb[:], lhsT=r(ones1[:]), rhs=r(rstd[:]), start=True, stop=True
                )
                rsb_sb = tps.tile([128, T], f32, tag="t512", name="t512")
                nc.vector.tensor_copy(rsb_sb[:], rsb[:])
                for dc in range(8):
                    t = tps.tile([128, T], f32, tag="t512", name="t512")
                    nc.vector.tensor_sub(t[:], q32(src[dc][:]), mub[:])
                    t2b = tps.tile([128, T], f32, tag="t512", name="t512")
                    nc.vector.tensor_mul(t2b[:], t[:], rsb_sb[:])
                    gt = ln_param(gsrc, l, dc)
                    bt = ln_param(bsrc, l, dc)
                    nc.scalar.activation(
                        dst[dc][:], t2b[:], AF.Identity, bias=bt[:], scale=gt[:]
                    )

            # ---------- Wo + residual + LN1 ----------
            for ec in range(8):
                pan = wpanel(I["woT"][l], slice(ec * 128, (ec + 1) * 128))
                ps = pp.tile([128, T], f32, tag="ps", name="ps")
                for dc in range(8):
                    nc.tensor.matmul(
                        out=ps[:], lhsT=r(pan[:, dc, :]), rhs=r(OP[dc][:]),
                        start=(dc == 0), stop=(dc == 7),
                    )
                bt = ln_param(I["bo"], l, ec)
                nc.vector.scalar_tensor_tensor(
                    out=X2[ec][:], in0=ps[:], scalar=bt[:], in1=q32(X[ec][:]),
                    op0=ALU.add, op1=ALU.add,
                )
            layernorm(X2, I["ln1g"], I["ln1b"], X)

            # ---------- FFN: dff blocked, W2 partials accumulated in SBUF ----------
            for blk in range(8):
                fbt = []
                for k in range(4):
                    fc = blk * 4 + k
                    pan = wpanel(I["w1T"][l], slice(fc * 128, (fc + 1) * 128))
                    ps = pp.tile([128, T], f32, tag="ps", name="ps")
                    for dc in range(8):
                        nc.tensor.matmul(
                            out=ps[:], lhsT=r(pan[:, dc, :]), rhs=r(X[dc][:]),
                            start=(dc == 0), stop=(dc == 7),
                        )
                    bt = ln_param(I["b1"], l, fc)
                    fb = fbp.tile([128, T], f32r, tag="fblk", name="fblk")
                    fbt.append(fb)
                    nc.scalar.activation(fb[:], ps[:], AF.Relu, bias=bt[:], scale=1.0)
                for ec in range(8):
                    pan = wp.tile([128, 4, 128], f32r, tag="wpan", name="wpan")
                    nc.sync.dma_start(
                        out=pan[:],
                        in_=I["w2T"][l, blk * 512 : (blk + 1) * 512,
                                     ec * 128 : (ec + 1) * 128].rearrange(
                            "(dc p) c -> p dc c", p=128
                        ),
                    )
                    ps = pp.tile([128, T], f32, tag="ps", name="ps")
                    for k in range(4):
                        nc.tensor.matmul(
                            out=ps[:], lhsT=r(pan[:, k, :]), rhs=r(fbt[k][:]),
                            start=(k == 0), stop=(k == 3),
                        )
                    if blk == 0:
                        bt = ln_param(I["b2"], l, ec)
                        nc.vector.scalar_tensor_tensor(
                            out=ACC[ec][:], in0=ps[:], scalar=bt[:], in1=q32(X[ec][:]),
                            op0=ALU.add, op1=ALU.add,
                        )
                    else:
                        nc.vector.tensor_add(ACC[ec][:], q32(ACC[ec][:]), ps[:])
            layernorm(ACC, I["ln2g"], I["ln2b"], X)

        for ec in range(8):
            nc.sync.dma_start(out=y[ec * 128 : (ec + 1) * 128, :], in_=q32(X[ec][:]))

    nc.compile()
    return nc


def _host_prep(inputs):
    g = {}
    Wqkv = np.asarray(inputs["Wqkv"], np.float32)
    bqkv = np.asarray(inputs["bqkv"], np.float32)
    sc = 1.0 / np.sqrt(HD)
    wqT = np.zeros((L, D, D), np.float32)
    wkT = np.zeros((L, D, D), np.float32)
    wvT = np.zeros((L, D + 1, V_E), np.float32)
    for l in range(L):
        Wq, Wk, Wv = Wqkv[l, 0:D], Wqkv[l, D : 2 * D], Wqkv[l, 2 * D :]
        bv = bqkv[l, 2 * D :]
        wqT[l] = Wq.T * sc
        wkT[l] = Wk.T
        for h in range(H):
            wvT[l, :D, h * 65 : h * 65 + 64] = Wv.T[:, h * 64 : h * 64 + 64]
            wvT[l, D, h * 65 : h * 65 + 64] = bv[h * 64 : h * 64 + 64]
    g["wqT"], g["wkT"], g["wvT"] = wqT, wkT, wvT
    g["woT"] = np.ascontiguousarray(np.asarray(inputs["Wo"], np.float32).transpose(0, 2, 1))
    g["w1T"] = np.ascontiguousarray(np.asarray(inputs["W1"], np.float32).transpose(0, 2, 1))
    g["w2T"] = np.ascontiguousarray(np.asarray(inputs["W2"], np.float32).transpose(0, 2, 1))
    g["bq"] = (bqkv[:, 0:D] * sc).reshape(L, 8, 128, 1)
    g["bk"] = bqkv[:, D : 2 * D].reshape(L, 8, 128, 1).copy()
    g["bo"] = np.asarray(inputs["bo"], np.float32).reshape(L, 8, 128, 1)
    g["b1"] = np.asarray(inputs["b1"], np.float32).reshape(L, 32, 128, 1)
    g["b2"] = np.asarray(inputs["b2"], np.float32).reshape(L, 8, 128, 1)
    g["ln1g"] = np.asarray(inputs["g1"], np.float32).reshape(L, 8, 128, 1)
    g["ln1b"] = np.asarray(inputs["be1"], np.float32).reshape(L, 8, 128, 1)
    g["ln2g"] = np.asarray(inputs["g2"], np.float32).reshape(L, 8, 128, 1)
    g["ln2b"] = np.asarray(inputs["be2"], np.float32).reshape(L, 8, 128, 1)
    g["ones1"] = np.ones((1, 128), np.float32)
    g["onesr"] = np.ones((1, 512), np.float32)
    g["onesd"] = np.full((128, 1), 1.0 / D, np.float32)

    xb = np.asarray(inputs["x"], np.float32).transpose(1, 0, 2)
    in_maps = []
    for c in range(8):
        b, hh = c // 2, c % 2
        pb = np.zeros((NC, 128, 1), np.float32)
        if hh == 0:
            pb[0:4] = NEG
            pb[8, 64:128] = NEG
        else:
            pb[8] = NEG
        m = dict(g)
        m["x0"] = np.ascontiguousarray(xb[b, hh * T : (hh + 1) * T, :].T)
        m["pb"] = pb
        in_maps.append(m)
    return in_maps


def kernel(**inputs):
    from concourse.bass_utils import run_bass_kernel_spmd

    if "nc" not in _CACHE:
        _CACHE["nc"] = _build_program()
    nc = _CACHE["nc"]
    in_maps = _host_prep(inputs)
    res = run_bass_kernel_spmd(nc, in_maps, core_ids=list(range(8)))
    out = np.zeros((S, B, D), np.float32)
    for c in range(8):
        b, hh = c // 2, c % 2
        out[hh * T : (hh + 1) * T, b, :] = res.results[c]["y"].T
    return out



# revision 4
# speedup vs baseline: 1.1950x; 1.1950x over previous
"""Trainium2 Bass kernel for nn_LocalTransformer (4-layer transformer,
d=1024, 16 heads, dff=4096, seq=1024, batch=4, causal + 64-lookahead mask).

Sharding: 8 cores = 4 samples x 2 sequence halves; each core owns 512
tokens. Attention context is a relative window of 1152 positions
(p = t - qoff + 512) so the mask predicate `p <= q + 576` is
core-independent; out-of-window positions are killed by per-core pad
biases added inside exp and boundary-chunk 0/1 mask multiplies.
K/V for remote positions move via a pairwise AllGather per layer; the
receiver computes peer = slot0 + slot1 - own (exact in fp32 for bf16).

Precision: all big matmuls in bf16 (weights host-converted, activations
shadowed in bf16) with fp32 PSUM accumulation; residual/LN in fp32.
bf16 weights enable fast-weight-load and LDW/MM overlap on the PE.
"""
import numpy as np

L, D, H, DFF, S, B = 4, 1024, 16, 4096, 1024, 4
HD = D // H  # 64
T = 512  # local tokens per core
WIN = 1152  # kv window positions (9 chunks of 128)
NC = 9
EPS = 1e-5
NEG = -30000.0
V_E = H * 65  # 1040: per head [V(64) | denominator-ones col]

_CACHE = {}

# packed per-layer bias/param columns in `ball` [L, 128, 96]
BQ, BK, BO, B2, L1G, L1B, L2G, L2B, B1 = 0, 8, 16, 24, 32, 40, 48, 56, 64


def _build_program():
    import concourse.bass as bass
    import concourse.tile as tile
    from concourse import bacc, mybir
    from contextlib import ExitStack

    f32, bf16, f32r = mybir.dt.float32, mybir.dt.bfloat16, mybir.dt.float32r
    AF = mybir.ActivationFunctionType
    ALU = mybir.AluOpType

    nc = bacc.Bacc("TRN2", target_bir_lowering=False, debug=False, num_devices=8)

    I = {}
    I["x0"] = nc.dram_tensor("x0", [D, T], f32r, kind="ExternalInput").ap()
    I["pb"] = nc.dram_tensor("pb", [NC, 128, 1], f32, kind="ExternalInput").ap()
    I["ball"] = nc.dram_tensor("ball", [L, 128, 96], f32, kind="ExternalInput").ap()
    for k, sh, dt in (
        ("wq", [L, 8, 128, 8, 128], bf16),
        ("wk", [L, 8, 128, 8, 128], bf16),
        ("wo", [L, 8, 128, 8, 128], bf16),
        ("w1", [L, 32, 128, 8, 128], bf16),
        ("w2", [L, 8, 8, 128, 4, 128], bf16),
        ("wv", [L, D + 1, V_E], bf16),
        ("selm", [5, 128, 512], bf16),
        ("onesr", [1, 512], bf16),
        ("ones1", [1, 128], f32r),
        ("onesd", [128, 1], f32r),
    ):
        I[k] = nc.dram_tensor(k, sh, dt, kind="ExternalInput").ap()
    y = nc.dram_tensor("y", [D, T], f32, kind="ExternalOutput").ap()

    cck_in, cck_out, ccv_in, ccv_out = [], [], [], []
    for l in range(L):
        cck_in.append(nc.dram_tensor(f"ccki{l}", [D, 576], bf16, kind="Internal").ap())
        cck_out.append(nc.dram_tensor(f"ccko{l}", [2 * D, 576], bf16, kind="Internal").ap())
        ccv_in.append(nc.dram_tensor(f"ccvi{l}", [576, V_E], bf16, kind="Internal").ap())
        ccv_out.append(nc.dram_tensor(f"ccvo{l}", [1152, V_E], bf16, kind="Internal").ap())

    RG = [[0, 1], [2, 3], [4, 5], [6, 7]]
    SIDX = {4: 0, 5: 1, 6: 2, 7: 3, 8: 4}
    HGROUPS = [range(0, 6), range(6, 12), range(12, 16)]

    with tile.TileContext(nc) as tc, ExitStack() as ctx:
        pers = ctx.enter_context(tc.tile_pool(name="pers", bufs=1))
        X = [pers.tile([128, T], f32r, tag=f"X{i}", name=f"X{i}") for i in range(8)]
        XB = [pers.tile([128, T], bf16, tag=f"XB{i}", name=f"XB{i}") for i in range(8)]
        X2 = [pers.tile([128, T], f32r, tag=f"X2{i}", name=f"X2{i}") for i in range(8)]
        OP = [pers.tile([128, T], bf16, tag=f"OP{i}", name=f"OP{i}") for i in range(8)]
        Q = [pers.tile([128, T], bf16, tag=f"Q{i}", name=f"Qt{i}") for i in range(8)]
        KH = [pers.tile([128, WIN], bf16, tag=f"KH{i}", name=f"KHt{i}") for i in range(8)]
        VT = [pers.tile([128, V_E], bf16, tag=f"VT{i}", name=f"VTt{i}") for i in range(NC)]
        ACC = [pers.tile([128, T], f32r, tag=f"ACC{i}", name=f"ACCt{i}") for i in range(8)]
        MT = [pers.tile([128, 512], bf16, tag=f"MT{i}", name=f"MTt{i}") for i in range(5)]
        ones_row = pers.tile([1, T], bf16, tag="ones_row", name="ones_row")
        ones1 = pers.tile([1, 128], f32r, tag="ones1", name="ones1t")
        onesd_t = pers.tile([128, 1], f32r, tag="onesd", name="onesdt")
        pb_t = [pers.tile([128, 1], f32, tag=f"pb{i}", name=f"pbt{i}") for i in range(NC)]
        D16 = pers.tile([16, T], f32, tag="D16", name="D16")
        R16 = pers.tile([16, T], f32, tag="R16", name="R16")

        wp = ctx.enter_context(tc.tile_pool(name="wp", bufs=6))  # weight panels
        smw = ctx.enter_context(tc.tile_pool(name="smw", bufs=3))  # small weights
        pp = ctx.enter_context(tc.tile_pool(name="pp", bufs=2, space="PSUM"))
        pav = ctx.enter_context(tc.tile_pool(name="pav", bufs=6, space="PSUM"))
        pr = ctx.enter_context(tc.tile_pool(name="pr", bufs=5))  # probs bf16
        tpv = ctx.enter_context(tc.tile_pool(name="tpv", bufs=2))  # [128,1040] f32
        tps = ctx.enter_context(tc.tile_pool(name="tps", bufs=3))  # [128,T] f32
        tpr = ctx.enter_context(tc.tile_pool(name="tpr", bufs=2))  # [128,T] f32r
        fx = ctx.enter_context(tc.tile_pool(name="fx", bufs=2))  # fixup bf16 loads
        sm = ctx.enter_context(tc.tile_pool(name="sm", bufs=3))  # [1,T] smalls
        fbp = ctx.enter_context(tc.tile_pool(name="fbp", bufs=8))  # FFN blocks
        rhp = ctx.enter_context(tc.tile_pool(name="rhp", bufs=2))  # [1,T] recip
        blp = ctx.enter_context(tc.tile_pool(name="blp", bufs=2))  # bias pack

        r = lambda ap: ap.bitcast(f32r)
        q32 = lambda ap: ap.bitcast(f32)

        nc.sync.dma_start(out=ones_row[:], in_=I["onesr"][:])
        nc.sync.dma_start(out=ones1[:], in_=I["ones1"][:])
        nc.sync.dma_start(out=onesd_t[:], in_=I["onesd"][:])
        for i in range(NC):
            nc.sync.dma_start(out=pb_t[i][:], in_=I["pb"][i])
        for i in range(5):
            nc.sync.dma_start(out=MT[i][:], in_=I["selm"][i])
        for i in range(8):
            nc.sync.dma_start(out=X[i][:], in_=I["x0"][i * 128 : (i + 1) * 128, :])
            nc.gpsimd.tensor_copy(XB[i][:], q32(X[i][:]))
        for i in range(8):
            nc.gpsimd.memset(KH[i][:], 0.0)
        for i in range(NC):
            nc.gpsimd.memset(VT[i][:], 0.0)

        def wpanel(name, l, idx):
            pan = wp.tile([128, 8, 128], bf16, tag="wpan", name="wpan")
            nc.sync.dma_start(out=pan[:], in_=I[name][l, idx])
            return pan

        for l in range(L):
            ball = blp.tile([128, 96], f32, tag="ball", name="ball")
            nc.sync.dma_start(out=ball[:], in_=I["ball"][l])
            bcol = lambda j: ball[:, j : j + 1]

            # ---------- K projection -> KH[:, 512:1024], stage, AllGather ----------
            for ec in range(8):
                pan = wpanel("wk", l, ec)
                ps = pp.tile([128, T], f32, tag="ps", name="ps")
                for dc in range(8):
                    nc.tensor.matmul(
                        out=ps[:], lhsT=pan[:, dc, :], rhs=XB[dc][:],
                        start=(dc == 0), stop=(dc == 7),
                    )
                nc.scalar.activation(
                    KH[ec][:, 512:1024], ps[:], AF.Identity, bias=bcol(BK + ec), scale=1.0
                )
                nc.sync.dma_start(
                    out=cck_in[l][ec * 128 : (ec + 1) * 128, 0:512],
                    in_=KH[ec][:, 512:1024],
                )
                nc.sync.dma_start(
                    out=cck_in[l][ec * 128 : (ec + 1) * 128, 512:576],
                    in_=KH[ec][:, 512:576],
                )
            nc.gpsimd.collective_compute(
                "AllGather", mybir.AluOpType.bypass, replica_groups=RG,
                ins=[cck_in[l][:]], outs=[cck_out[l][:]],
            )

            # ---------- V projection -> VT[4..7], stage, AllGather ----------
            for tcx in range(4):
                for s0, s1 in ((0, 512), (512, 1024), (1024, V_E)):
                    w = s1 - s0
                    ps = pp.tile([128, T], f32, tag="ps", name="ps")
                    for dc in range(8):
                        pan = smw.tile([128, 512], bf16, tag="vpan", name="vpan")
                        nc.sync.dma_start(
                            out=pan[:, 0:w],
                            in_=I["wv"][l, dc * 128 : (dc + 1) * 128, s0:s1],
                        )
                        nc.tensor.matmul(
                            out=ps[:, 0:w],
                            lhsT=XB[dc][:, tcx * 128 : (tcx + 1) * 128],
                            rhs=pan[:, 0:w],
                            start=(dc == 0), stop=False,
                        )
                    bia = smw.tile([1, 512], bf16, tag="vbias", name="vbias")
                    nc.sync.dma_start(out=bia[:, 0:w], in_=I["wv"][l, D : D + 1, s0:s1])
                    nc.tensor.matmul(
                        out=ps[:, 0:w],
                        lhsT=ones_row[:, 0:128],
                        rhs=bia[:, 0:w],
                        start=False, stop=True,
                    )
                    nc.scalar.activation(VT[4 + tcx][:, s0:s1], ps[:, 0:w], AF.Copy)
                nc.sync.dma_start(
                    out=ccv_in[l][tcx * 128 : (tcx + 1) * 128, :], in_=VT[4 + tcx][:]
                )
            nc.sync.dma_start(out=ccv_in[l][512:576, :], in_=VT[4][0:64, :])
            nc.gpsimd.collective_compute(
                "AllGather", mybir.AluOpType.bypass, replica_groups=RG,
                ins=[ccv_in[l][:]], outs=[ccv_out[l][:]],
            )
            for pc in range(4, 8):  # local denominator ones-columns
                nc.gpsimd.memset(
                    VT[pc][:].rearrange("p (h c) -> p h c", c=65)[:, :, 64:65], 1.0
                )

            # ---------- Q projection ----------
            for ec in range(8):
                pan = wpanel("wq", l, ec)
                ps = pp.tile([128, T], f32, tag="ps", name="ps")
                for dc in range(8):
                    nc.tensor.matmul(
                        out=ps[:], lhsT=pan[:, dc, :], rhs=XB[dc][:],
                        start=(dc == 0), stop=(dc == 7),
                    )
                nc.scalar.activation(
                    Q[ec][:], ps[:], AF.Identity, bias=bcol(BQ + ec), scale=1.0
                )

            # ---------- remote KV fixup (consumes AllGather results) ----------
            for ec in range(8):
                ta = fx.tile([128, 576], bf16, tag="fxa", name="fxa")
                tb = fx.tile([128, 576], bf16, tag="fxb", name="fxb")
                nc.sync.dma_start(out=ta[:], in_=cck_out[l][ec * 128 : (ec + 1) * 128, :])
                nc.sync.dma_start(
                    out=tb[:], in_=cck_out[l][D + ec * 128 : D + (ec + 1) * 128, :]
                )
                ts = tpv.tile([128, 1040], f32, tag="fxv", name="fxv")
                nc.vector.tensor_add(ts[:, 0:576], ta[:], tb[:])
                nc.vector.tensor_sub(
                    KH[ec][:, 0:512], ts[:, 0:512], KH[ec][:, 512:1024]
                )
                nc.vector.tensor_sub(
                    KH[ec][:, 1024:1088], ts[:, 512:576], KH[ec][:, 512:576]
                )
            for pc in range(4):
                ta = fx.tile([128, V_E], bf16, tag="fxa", name="fxa")
                tb = fx.tile([128, V_E], bf16, tag="fxb", name="fxb")
                nc.sync.dma_start(out=ta[:], in_=ccv_out[l][pc * 128 : (pc + 1) * 128, :])
                nc.sync.dma_start(
                    out=tb[:], in_=ccv_out[l][576 + pc * 128 : 576 + (pc + 1) * 128, :]
                )
                ts = tpv.tile([128, 1040], f32, tag="fxv", name="fxv")
                nc.vector.tensor_add(ts[:], ta[:], tb[:])
                nc.vector.tensor_sub(VT[pc][:], ts[:], VT[4 + pc][:])
            ta = fx.tile([128, V_E], bf16, tag="fxa", name="fxa")
            tb = fx.tile([128, V_E], bf16, tag="fxb", name="fxb")
            nc.sync.dma_start(out=ta[0:64, :], in_=ccv_out[l][512:576, :])
            nc.sync.dma_start(out=tb[0:64, :], in_=ccv_out[l][1088:1152, :])
            ts = tpv.tile([128, 1040], f32, tag="fxv", name="fxv")
            nc.vector.tensor_add(ts[0:64, :], ta[0:64, :], tb[0:64, :])
            nc.vector.tensor_sub(VT[8][0:64, :], ts[0:64, :], VT[4][0:64, :])
            for pc in (0, 1, 2, 3, 8):
                nc.gpsimd.memset(
                    VT[pc][:].rearrange("p (h c) -> p h c", c=65)[:, :, 64:65], 1.0
                )

            # ---------- attention, head groups with batched reciprocal ----------
            avps = {}
            for grp in HGROUPS:
                for h in grp:
                    par, kc = h % 2, h // 2
                    rows = slice(par * 64, par * 64 + 64)
                    av = pav.tile([128, T], f32, tag="av", name="av")
                    avps[h] = av
                    for idx, c in enumerate([4, 5, 6, 7, 0, 1, 2, 3, 8]):
                        sc = pp.tile([128, T], f32, tag="ps", name="ps")
                        nc.tensor.matmul(
                            out=sc[:],
                            lhsT=KH[kc][rows, c * 128 : (c + 1) * 128],
                            rhs=Q[kc][rows, :],
                            start=True, stop=True,
                        )
                        pt = pr.tile([128, T], bf16, tag="probs", name="probs")
                        nc.scalar.activation(
                            pt[:], sc[:], AF.Exp, bias=pb_t[c][:], scale=1.0
                        )
                        if c in SIDX:
                            nc.vector.tensor_mul(pt[:], pt[:], MT[SIDX[c]][:])
                        nc.tensor.matmul(
                            out=av[0:65, :],
                            lhsT=VT[c][:, h * 65 : h * 65 + 65],
                            rhs=pt[:],
                            start=(idx == 0), stop=(idx == 8),
                        )
                    dt_ = tps.tile([128, T], f32, tag="t512", name="t512")
                    nc.vector.tensor_copy(dt_[64:65, :], av[64:65, :])
                    j = h - grp.start
                    nc.sync.dma_start(out=D16[j : j + 1, :], in_=dt_[64:65, :])
                gs = grp.stop - grp.start
                nc.vector.reciprocal(R16[0:gs, :], D16[0:gs, :])
                for h in grp:
                    j = h - grp.start
                    rht = rhp.tile([1, T], f32r, tag="rh", name="rht")
                    nc.sync.dma_start(out=rht[:], in_=R16[j : j + 1, :].bitcast(f32r))
                    bc = pp.tile([128, T], f32, tag="ps", name="ps")
                    nc.tensor.matmul(
                        out=bc[0:64, :], lhsT=r(ones1[:, 0:64]), rhs=r(rht[:]),
                        start=True, stop=True,
                    )
                    bcs = tps.tile([128, T], f32, tag="t512", name="t512")
                    nc.scalar.activation(bcs[0:64, :], bc[0:64, :], AF.Copy)
                    if h % 2 == 0:
                        nc.vector.tensor_mul(
                            OP[h // 2][0:64, :], avps[h][0:64, :], bcs[0:64, :]
                        )
                    else:
                        ot = pr.tile([128, T], bf16, tag="otb", name="otb")
                        nc.vector.tensor_mul(
                            ot[0:64, :], avps[h][0:64, :], bcs[0:64, :]
                        )
                        nc.sync.dma_start(
                            out=OP[h // 2][64:128, :], in_=ot[0:64, :]
                        )

            def layernorm(src, gcol, bc_, dst):
                mu = pp.tile([1, T], f32, tag="ps", name="ps")
                ms = pp.tile([1, T], f32, tag="ps", name="ps")
                for dc in range(8):
                    sq = tpr.tile([128, T], f32r, tag="sqr", name="sqr")
                    nc.scalar.square(sq[:], q32(src[dc][:]))
                    nc.tensor.matmul(
                        out=mu[:], lhsT=r(onesd_t[:]), rhs=r(src[dc][:]),
                        start=(dc == 0), stop=(dc == 7),
                    )
                    nc.tensor.matmul(
                        out=ms[:], lhsT=r(onesd_t[:]), rhs=r(sq[:]),
                        start=(dc == 0), stop=(dc == 7),
                    )
                mu_sb = sm.tile([1, T], f32r, tag="sm1", name="mu")
                nc.vector.tensor_copy(mu_sb[:], mu[:])
                t2 = sm.tile([1, T], f32, tag="sm1", name="t2")
                nc.vector.tensor_mul(t2[:], q32(mu_sb[:]), q32(mu_sb[:]))
                var = sm.tile([1, T], f32, tag="sm1", name="var")
                nc.vector.tensor_sub(var[:], ms[:], t2[:])
                nc.vector.tensor_scalar_add(var[:], var[:], EPS)
                std = sm.tile([1, T], f32, tag="sm1", name="std")
                nc.scalar.sqrt(std[:], var[:])
                rstd = sm.tile([1, T], f32r, tag="sm1", name="rstd")
                with nc.allow_low_precision(reason="f32r is fp32-width storage"):
                    nc.vector.reciprocal(rstd[:], std[:])
                mub = pp.tile([128, T], f32, tag="ps", name="ps")
                nc.tensor.matmul(
                    out=mub[:], lhsT=r(ones1[:]), rhs=r(mu_sb[:]), start=True, stop=True
                )
                rsb = pp.tile([128, T], f32, tag="ps", name="ps")
                nc.tensor.matmul(
                    out=rsb[:], lhsT=r(ones1[:]), rhs=r(rstd[:]), start=True, stop=True
                )
                rsb_sb = tps.tile([128, T], f32, tag="t512", name="t512")
                nc.vector.tensor_copy(rsb_sb[:], rsb[:])
                for dc in range(8):
                    t = tps.tile([128, T], f32, tag="t512", name="t512")
                    nc.vector.tensor_sub(t[:], q32(src[dc][:]), mub[:])
                    t2b = tps.tile([128, T], f32, tag="t512", name="t512")
                    nc.vector.tensor_mul(t2b[:], t[:], rsb_sb[:])
                    nc.scalar.activation(
                        dst[dc][:], t2b[:], AF.Identity,
                        bias=bcol(bc_ + dc), scale=bcol(gcol + dc),
                    )
                    nc.gpsimd.tensor_copy(XB[dc][:], q32(dst[dc][:]))

            # ---------- Wo + residual + LN1 ----------
            for ec in range(8):
                pan = wpanel("wo", l, ec)
                ps = pp.tile([128, T], f32, tag="ps", name="ps")
                for dc in range(8):
                    nc.tensor.matmul(
                        out=ps[:], lhsT=pan[:, dc, :], rhs=OP[dc][:],
                        start=(dc == 0), stop=(dc == 7),
                    )
                nc.vector.scalar_tensor_tensor(
                    out=X2[ec][:], in0=ps[:], scalar=bcol(BO + ec), in1=q32(X[ec][:]),
                    op0=ALU.add, op1=ALU.add,
                )
            layernorm(X2, L1G, L1B, X)

            # ---------- FFN: dff blocked, W2 partials accumulated in SBUF ----------
            for blk in range(8):
                fbt = []
                for k in range(4):
                    fc = blk * 4 + k
                    pan = wpanel("w1", l, fc)
                    ps = pp.tile([128, T], f32, tag="ps", name="ps")
                    for dc in range(8):
                        nc.tensor.matmul(
                            out=ps[:], lhsT=pan[:, dc, :], rhs=XB[dc][:],
                            start=(dc == 0), stop=(dc == 7),
                        )
                    fb = fbp.tile([128, T], bf16, tag="fblk", name="fblk")
                    fbt.append(fb)
                    nc.scalar.activation(
                        fb[:], ps[:], AF.Relu, bias=bcol(B1 + fc), scale=1.0
                    )
                for ec in range(8):
                    pan = wp.tile([128, 4, 128], bf16, tag="wpan", name="wpan")
                    nc.sync.dma_start(out=pan[:], in_=I["w2"][l, blk, ec])
                    ps = pp.tile([128, T], f32, tag="ps", name="ps")
                    for k in range(4):
                        nc.tensor.matmul(
                            out=ps[:], lhsT=pan[:, k, :], rhs=fbt[k][:],
                            start=(k == 0), stop=(k == 3),
                        )
                    if blk == 0:
                        nc.vector.scalar_tensor_tensor(
                            out=ACC[ec][:], in0=ps[:], scalar=bcol(B2 + ec),
                            in1=q32(X[ec][:]), op0=ALU.add, op1=ALU.add,
                        )
                    else:
                        nc.vector.tensor_add(ACC[ec][:], q32(ACC[ec][:]), ps[:])
            layernorm(ACC, L2G, L2B, X)

        for ec in range(8):
            nc.sync.dma_start(out=y[ec * 128 : (ec + 1) * 128, :], in_=q32(X[ec][:]))

    nc.compile()
    return nc


def _host_prep(inputs):
    import ml_dtypes

    bf16 = ml_dtypes.bfloat16
    g = {}
    Wqkv = np.asarray(inputs["Wqkv"], np.float32)
    bqkv = np.asarray(inputs["bqkv"], np.float32)
    sc = 1.0 / np.sqrt(HD)

    def panel8(wT):  # [L, 1024, 1024] -> [L, 8, 128, 8, 128]
        return np.ascontiguousarray(
            wT.reshape(L, 8, 128, 8, 128).transpose(0, 3, 2, 1, 4)
        ).astype(bf16)

    Wq = Wqkv[:, 0:D]  # [L, D, D] (out, in)
    Wk = Wqkv[:, D : 2 * D]
    Wv = Wqkv[:, 2 * D :]
    bv = bqkv[:, 2 * D :]
    wqT = np.ascontiguousarray(Wq.transpose(0, 2, 1)) * sc
    wkT = np.ascontiguousarray(Wk.transpose(0, 2, 1))
    g["wq"] = panel8(wqT)
    g["wk"] = panel8(wkT)
    g["wo"] = panel8(np.ascontiguousarray(np.asarray(inputs["Wo"], np.float32).transpose(0, 2, 1)))
    w1T = np.ascontiguousarray(np.asarray(inputs["W1"], np.float32).transpose(0, 2, 1))
    g["w1"] = np.ascontiguousarray(
        w1T.reshape(L, 8, 128, 32, 128).transpose(0, 3, 2, 1, 4)
    ).astype(bf16)
    w2T = np.ascontiguousarray(np.asarray(inputs["W2"], np.float32).transpose(0, 2, 1))
    g["w2"] = np.ascontiguousarray(
        w2T.reshape(L, 8, 4, 128, 8, 128).transpose(0, 1, 4, 3, 2, 5)
    ).astype(bf16)

    wv = np.zeros((L, D + 1, V_E), np.float32)
    for l in range(L):
        WvT = Wv[l].T
        for h in range(H):
            wv[l, :D, h * 65 : h * 65 + 64] = WvT[:, h * 64 : h * 64 + 64]
            wv[l, D, h * 65 : h * 65 + 64] = bv[l, h * 64 : h * 64 + 64]
    g["wv"] = wv.astype(bf16)

    ball = np.zeros((L, 128, 96), np.float32)
    ball[:, :, BQ : BQ + 8] = (bqkv[:, 0:D] * sc).reshape(L, 8, 128).transpose(0, 2, 1)
    ball[:, :, BK : BK + 8] = bqkv[:, D : 2 * D].reshape(L, 8, 128).transpose(0, 2, 1)
    ball[:, :, BO : BO + 8] = np.asarray(inputs["bo"], np.float32).reshape(L, 8, 128).transpose(0, 2, 1)
    ball[:, :, B2 : B2 + 8] = np.asarray(inputs["b2"], np.float32).reshape(L, 8, 128).transpose(0, 2, 1)
    ball[:, :, L1G : L1G + 8] = np.asarray(inputs["g1"], np.float32).reshape(L, 8, 128).transpose(0, 2, 1)
    ball[:, :, L1B : L1B + 8] = np.asarray(inputs["be1"], np.float32).reshape(L, 8, 128).transpose(0, 2, 1)
    ball[:, :, L2G : L2G + 8] = np.asarray(inputs["g2"], np.float32).reshape(L, 8, 128).transpose(0, 2, 1)
    ball[:, :, L2B : L2B + 8] = np.asarray(inputs["be2"], np.float32).reshape(L, 8, 128).transpose(0, 2, 1)
    ball[:, :, B1 : B1 + 32] = np.asarray(inputs["b1"], np.float32).reshape(L, 32, 128).transpose(0, 2, 1)
    g["ball"] = ball

    # boundary masks for chunks 4..8: keep iff c*128 + p <= q + 576
    selm = np.zeros((5, 128, 512), np.float32)
    p = np.arange(128)[:, None]
    q = np.arange(512)[None, :]
    for i, c in enumerate((4, 5, 6, 7, 8)):
        selm[i] = (c * 128 + p <= q + 576).astype(np.float32)
    g["selm"] = selm.astype(bf16)

    g["onesr"] = np.ones((1, 512), np.float32).astype(bf16)
    g["ones1"] = np.ones((1, 128), np.float32)
    g["onesd"] = np.full((128, 1), 1.0 / D, np.float32)

    xb = np.asarray(inputs["x"], np.float32).transpose(1, 0, 2)
    in_maps = []
    for c in range(8):
        b, hh = c // 2, c % 2
        pb = np.zeros((NC, 128, 1), np.float32)
        if hh == 0:
            pb[0:4] = NEG
            pb[8, 64:128] = NEG
        else:
            pb[8] = NEG
        m = dict(g)
        m["x0"] = np.ascontiguousarray(xb[b, hh * T : (hh + 1) * T, :].T)
        m["pb"] = pb
        in_maps.append(m)
    return in_maps


def kernel(**inputs):
    from concourse.bass_utils import run_bass_kernel_spmd

    if "nc" not in _CACHE:
        _CACHE["nc"] = _build_program()
    nc = _CACHE["nc"]
    in_maps = _host_prep(inputs)
    res = run_bass_kernel_spmd(nc, in_maps, core_ids=list(range(8)))
    out = np.zeros((S, B, D), np.float32)
    for c in range(8):
        b, hh = c // 2, c % 2
        out[hh * T : (hh + 1) * T, b, :] = res.results[c]["y"].T
    return out


# revision 14
# speedup vs baseline: 1.2153x; 1.0170x over previous
"""Trainium2 Bass kernel for nn_LocalTransformer (4-layer transformer,
d=1024, 16 heads, dff=4096, seq=1024, batch=4, causal + 64-lookahead mask).

Sharding: 8 cores = 4 samples x 2 sequence halves; each core owns 512
tokens. Attention context is a relative window of 1152 positions
(p = t - qoff + 512) so the mask predicate `p <= q + 576` is
core-independent; out-of-window positions are killed by per-core pad
biases added inside exp and boundary-chunk 0/1 mask multiplies.
K/V for remote positions move via a pairwise AllGather per layer; the
receiver computes peer = slot0 + slot1 - own (exact in fp32 for bf16).

Precision: all big matmuls in bf16 (weights host-converted, activations
shadowed in bf16) with fp32 PSUM accumulation; residual/LN in fp32.
bf16 weights enable fast-weight-load and LDW/MM overlap on the PE.
"""
import numpy as np

L, D, H, DFF, S, B = 4, 1024, 16, 4096, 1024, 4
HD = D // H  # 64
T = 512  # local tokens per core
WIN = 1152  # kv window positions (9 chunks of 128)
NC = 9
EPS = 1e-5
NEG = -30000.0
V_E = H * 65  # 1040: per head [V(64) | denominator-ones col]

_CACHE = {}

# packed per-layer bias/param columns in `ball` [L, 128, 96]
BQ, BK, BO, B2, L1G, L1B, L2G, L2B, B1 = 0, 8, 16, 24, 32, 40, 48, 56, 64


def _build_program():
    import concourse.bass as bass
    import concourse.tile as tile
    from concourse import bacc, mybir
    from contextlib import ExitStack

    f32, bf16, f32r = mybir.dt.float32, mybir.dt.bfloat16, mybir.dt.float32r
    AF = mybir.ActivationFunctionType
    ALU = mybir.AluOpType

    nc = bacc.Bacc("TRN2", target_bir_lowering=False, debug=False, num_devices=8)

    I = {}
    I["x0"] = nc.dram_tensor("x0", [D, T], bf16, kind="ExternalInput").ap()
    I["pb"] = nc.dram_tensor("pb", [NC, 128, 1], f32, kind="ExternalInput").ap()
    I["ball"] = nc.dram_tensor("ball", [L, 128, 96], f32, kind="ExternalInput").ap()
    for k, sh, dt in (
        ("wq", [L, 8, 128, 8, 128], bf16),
        ("wk", [L, 8, 128, 8, 128], bf16),
        ("wo", [L, 8, 128, 8, 128], bf16),
        ("w1", [L, 32, 128, 8, 128], bf16),
        ("w2", [L, 8, 8, 128, 4, 128], bf16),
        ("wv", [L, D + 1, V_E], bf16),
        ("selm", [5, 128, 512], bf16),
        ("onesr", [1, 512], bf16),
        ("ones1", [1, 128], f32r),
        ("onesd", [128, 1], f32r),
    ):
        I[k] = nc.dram_tensor(k, sh, dt, kind="ExternalInput").ap()
    y = nc.dram_tensor("y", [D, T], bf16, kind="ExternalOutput").ap()

    cck_in, cck_out, ccv_in, ccv_out = [], [], [], []
    for l in range(L):
        cck_in.append(nc.dram_tensor(f"ccki{l}", [D, 576], bf16, kind="Internal").ap())
        cck_out.append(nc.dram_tensor(f"ccko{l}", [2 * D, 576], bf16, kind="Internal").ap())
        ccv_in.append(nc.dram_tensor(f"ccvi{l}", [576, V_E], bf16, kind="Internal").ap())
        ccv_out.append(nc.dram_tensor(f"ccvo{l}", [1152, V_E], bf16, kind="Internal").ap())

    RG = [[0, 1], [2, 3], [4, 5], [6, 7]]
    SIDX = {4: 0, 5: 1, 6: 2, 7: 3, 8: 4}
    HGROUPS = [range(0, 6), range(6, 12), range(12, 16)]

    with tile.TileContext(nc) as tc, ExitStack() as ctx:
        pers = ctx.enter_context(tc.tile_pool(name="pers", bufs=1))
        XB = [pers.tile([128, T], bf16, tag=f"XB{i}", name=f"XB{i}") for i in range(8)]
        X2 = [pers.tile([128, T], f32r, tag=f"X2{i}", name=f"X2{i}") for i in range(8)]
        OP = [pers.tile([128, T], bf16, tag=f"OP{i}", name=f"OP{i}") for i in range(8)]
        Q = [pers.tile([128, T], bf16, tag=f"Q{i}", name=f"Qt{i}") for i in range(8)]
        KH = [pers.tile([128, WIN], bf16, tag=f"KH{i}", name=f"KHt{i}") for i in range(8)]
        VT = [pers.tile([128, V_E], bf16, tag=f"VT{i}", name=f"VTt{i}") for i in range(NC)]
        ACC = [pers.tile([128, T], f32r, tag=f"ACC{i}", name=f"ACCt{i}") for i in range(8)]
        MT = [pers.tile([128, 512], bf16, tag=f"MT{i}", name=f"MTt{i}") for i in range(5)]
        ones_row = pers.tile([1, T], bf16, tag="ones_row", name="ones_row")
        ones1 = pers.tile([1, 128], f32r, tag="ones1", name="ones1t")
        onesd_t = pers.tile([128, 1], f32r, tag="onesd", name="onesdt")
        pb_t = [pers.tile([128, 1], f32, tag=f"pb{i}", name=f"pbt{i}") for i in range(NC)]
        D16 = pers.tile([16, T], f32, tag="D16", name="D16")
        R16 = pers.tile([16, T], f32, tag="R16", name="R16")

        wp = ctx.enter_context(tc.tile_pool(name="wp", bufs=6))  # weight panels
        smw = ctx.enter_context(tc.tile_pool(name="smw", bufs=3))  # small weights
        pp = ctx.enter_context(tc.tile_pool(name="pp", bufs=2, space="PSUM"))
        pav = ctx.enter_context(tc.tile_pool(name="pav", bufs=6, space="PSUM"))
        pr = ctx.enter_context(tc.tile_pool(name="pr", bufs=5))  # probs bf16
        tpv = ctx.enter_context(tc.tile_pool(name="tpv", bufs=2))  # [128,1040] f32
        tps = ctx.enter_context(tc.tile_pool(name="tps", bufs=3))  # [128,T] f32
        tpr = ctx.enter_context(tc.tile_pool(name="tpr", bufs=2))  # [128,T] f32r
        fx = ctx.enter_context(tc.tile_pool(name="fx", bufs=2))  # fixup bf16 loads
        sm = ctx.enter_context(tc.tile_pool(name="sm", bufs=3))  # [1,T] smalls
        fbp = ctx.enter_context(tc.tile_pool(name="fbp", bufs=8))  # FFN blocks
        rhp = ctx.enter_context(tc.tile_pool(name="rhp", bufs=2))  # [1,T] recip
        blp = ctx.enter_context(tc.tile_pool(name="blp", bufs=2))  # bias pack

        r = lambda ap: ap.bitcast(f32r)
        q32 = lambda ap: ap.bitcast(f32)

        nc.sync.dma_start(out=ones_row[:], in_=I["onesr"][:])
        nc.sync.dma_start(out=ones1[:], in_=I["ones1"][:])
        nc.sync.dma_start(out=onesd_t[:], in_=I["onesd"][:])
        for i in range(NC):
            nc.sync.dma_start(out=pb_t[i][:], in_=I["pb"][i])
        for i in range(5):
            nc.sync.dma_start(out=MT[i][:], in_=I["selm"][i])
        for i in range(8):
            nc.sync.dma_start(out=XB[i][:], in_=I["x0"][i * 128 : (i + 1) * 128, :])
        for i in range(8):
            nc.gpsimd.memset(KH[i][:], 0.0)
        for i in range(NC):
            nc.gpsimd.memset(VT[i][:], 0.0)

        def wpanel(name, l, idx):
            pan = wp.tile([128, 8, 128], bf16, tag="wpan", name="wpan")
            nc.sync.dma_start(out=pan[:], in_=I[name][l, idx])
            return pan

        for l in range(L):
            ball = blp.tile([128, 96], f32, tag="ball", name="ball")
            nc.sync.dma_start(out=ball[:], in_=I["ball"][l])
            bcol = lambda j: ball[:, j : j + 1]

            # ---------- K projection -> KH[:, 512:1024], stage, AllGather ----------
            for ec in range(8):
                pan = wpanel("wk", l, ec)
                ps = pp.tile([128, T], f32, tag="ps", name="ps")
                for dc in range(8):
                    nc.tensor.matmul(
                        out=ps[:], lhsT=pan[:, dc, :], rhs=XB[dc][:],
                        start=(dc == 0), stop=(dc == 7),
                    )
                nc.scalar.activation(
                    KH[ec][:, 512:1024], ps[:], AF.Identity, bias=bcol(BK + ec), scale=1.0
                )
                nc.sync.dma_start(
                    out=cck_in[l][ec * 128 : (ec + 1) * 128, 0:512],
                    in_=KH[ec][:, 512:1024],
                )
                nc.sync.dma_start(
                    out=cck_in[l][ec * 128 : (ec + 1) * 128, 512:576],
                    in_=KH[ec][:, 512:576],
                )
            nc.gpsimd.collective_compute(
                "AllGather", mybir.AluOpType.bypass, replica_groups=RG,
                ins=[cck_in[l][:]], outs=[cck_out[l][:]],
            )

            # ---------- V projection -> VT[4..7], stage, AllGather ----------
            for tcx in range(4):
                for s0, s1 in ((0, 512), (512, 1024), (1024, V_E)):
                    w = s1 - s0
                    ps = pp.tile([128, T], f32, tag="ps", name="ps")
                    for dc in range(8):
                        pan = smw.tile([128, 512], bf16, tag="vpan", name="vpan")
                        nc.sync.dma_start(
                            out=pan[:, 0:w],
                            in_=I["wv"][l, dc * 128 : (dc + 1) * 128, s0:s1],
                        )
                        nc.tensor.matmul(
                            out=ps[:, 0:w],
                            lhsT=XB[dc][:, tcx * 128 : (tcx + 1) * 128],
                            rhs=pan[:, 0:w],
                            start=(dc == 0), stop=False,
                        )
                    bia = smw.tile([1, 512], bf16, tag="vbias", name="vbias")
                    nc.sync.dma_start(out=bia[:, 0:w], in_=I["wv"][l, D : D + 1, s0:s1])
                    nc.tensor.matmul(
                        out=ps[:, 0:w],
                        lhsT=ones_row[:, 0:128],
                        rhs=bia[:, 0:w],
                        start=False, stop=True,
                    )
                    nc.scalar.activation(VT[4 + tcx][:, s0:s1], ps[:, 0:w], AF.Copy)
                nc.sync.dma_start(
                    out=ccv_in[l][tcx * 128 : (tcx + 1) * 128, :], in_=VT[4 + tcx][:]
                )
            nc.sync.dma_start(out=ccv_in[l][512:576, :], in_=VT[4][0:64, :])
            nc.gpsimd.collective_compute(
                "AllGather", mybir.AluOpType.bypass, replica_groups=RG,
                ins=[ccv_in[l][:]], outs=[ccv_out[l][:]],
            )
            for pc in range(4, 8):  # local denominator ones-columns
                nc.gpsimd.memset(
                    VT[pc][:].rearrange("p (h c) -> p h c", c=65)[:, :, 64:65], 1.0
                )

            # ---------- Q projection ----------
            for ec in range(8):
                pan = wpanel("wq", l, ec)
                ps = pp.tile([128, T], f32, tag="ps", name="ps")
                for dc in range(8):
                    nc.tensor.matmul(
                        out=ps[:], lhsT=pan[:, dc, :], rhs=XB[dc][:],
                        start=(dc == 0), stop=(dc == 7),
                    )
                nc.scalar.activation(
                    Q[ec][:], ps[:], AF.Identity, bias=bcol(BQ + ec), scale=1.0
                )

            # ---------- remote KV fixup (consumes AllGather results) ----------
            for ec in range(8):
                ta = fx.tile([128, 576], bf16, tag="fxa", name="fxa")
                tb = fx.tile([128, 576], bf16, tag="fxb", name="fxb")
                nc.sync.dma_start(out=ta[:], in_=cck_out[l][ec * 128 : (ec + 1) * 128, :])
                nc.sync.dma_start(
                    out=tb[:], in_=cck_out[l][D + ec * 128 : D + (ec + 1) * 128, :]
                )
                ts = tpv.tile([128, 1040], f32, tag="fxv", name="fxv")
                nc.vector.tensor_add(ts[:, 0:576], ta[:], tb[:])
                nc.vector.tensor_sub(
                    KH[ec][:, 0:512], ts[:, 0:512], KH[ec][:, 512:1024]
                )
                nc.vector.tensor_sub(
                    KH[ec][:, 1024:1088], ts[:, 512:576], KH[ec][:, 512:576]
                )
            for pc in range(4):
                ta = fx.tile([128, V_E], bf16, tag="fxa", name="fxa")
                tb = fx.tile([128, V_E], bf16, tag="fxb", name="fxb")
                nc.sync.dma_start(out=ta[:], in_=ccv_out[l][pc * 128 : (pc + 1) * 128, :])
                nc.sync.dma_start(
                    out=tb[:], in_=ccv_out[l][576 + pc * 128 : 576 + (pc + 1) * 128, :]
                )
                ts = tpv.tile([128, 1040], f32, tag="fxv", name="fxv")
                nc.vector.tensor_add(ts[:], ta[:], tb[:])
                nc.vector.tensor_sub(VT[pc][:], ts[:], VT[4 + pc][:])
            ta = fx.tile([128, V_E], bf16, tag="fxa", name="fxa")
            tb = fx.tile([128, V_E], bf16, tag="fxb", name="fxb")
            nc.sync.dma_start(out=ta[0:64, :], in_=ccv_out[l][512:576, :])
            nc.sync.dma_start(out=tb[0:64, :], in_=ccv_out[l][1088:1152, :])
            ts = tpv.tile([128, 1040], f32, tag="fxv", name="fxv")
            nc.vector.tensor_add(ts[0:64, :], ta[0:64, :], tb[0:64, :])
            nc.vector.tensor_sub(VT[8][0:64, :], ts[0:64, :], VT[4][0:64, :])
            for pc in (0, 1, 2, 3, 8):
                nc.gpsimd.memset(
                    VT[pc][:].rearrange("p (h c) -> p h c", c=65)[:, :, 64:65], 1.0
                )

            # ---------- attention, head groups with batched reciprocal ----------
            avps = {}
            for grp in HGROUPS:
                for h in grp:
                    par, kc = h % 2, h // 2
                    rows = slice(par * 64, par * 64 + 64)
                    av = pav.tile([128, T], f32, tag="av", name="av")
                    avps[h] = av
                    for idx, c in enumerate([4, 5, 6, 7, 0, 1, 2, 3, 8]):
                        sc = pp.tile([128, T], f32, tag="ps", name="ps")
                        nc.tensor.matmul(
                            out=sc[:],
                            lhsT=KH[kc][rows, c * 128 : (c + 1) * 128],
                            rhs=Q[kc][rows, :],
                            start=True, stop=True,
                        )
                        pt = pr.tile([128, T], bf16, tag="probs", name="probs")
                        nc.scalar.activation(
                            pt[:], sc[:], AF.Exp, bias=pb_t[c][:], scale=1.0
                        )
                        if c in SIDX:
                            nc.vector.tensor_mul(pt[:], pt[:], MT[SIDX[c]][:])
                        nc.tensor.matmul(
                            out=av[0:65, :],
                            lhsT=VT[c][:, h * 65 : h * 65 + 65],
                            rhs=pt[:],
                            start=(idx == 0), stop=(idx == 8),
                        )
                    dt_ = tps.tile([128, T], f32, tag="t512", name="t512")
                    nc.vector.tensor_copy(dt_[64:65, :], av[64:65, :])
                    j = h - grp.start
                    nc.sync.dma_start(out=D16[j : j + 1, :], in_=dt_[64:65, :])
                gs = grp.stop - grp.start
                nc.vector.reciprocal(R16[0:gs, :], D16[0:gs, :])
                for h in grp:
                    j = h - grp.start
                    rht = rhp.tile([1, T], f32r, tag="rh", name="rht")
                    nc.sync.dma_start(out=rht[:], in_=R16[j : j + 1, :].bitcast(f32r))
                    bc = pp.tile([128, T], f32, tag="ps", name="ps")
                    nc.tensor.matmul(
                        out=bc[0:64, :], lhsT=r(ones1[:, 0:64]), rhs=r(rht[:]),
                        start=True, stop=True,
                    )
                    bcs = tps.tile([128, T], f32, tag="t512", name="t512")
                    nc.scalar.activation(bcs[0:64, :], bc[0:64, :], AF.Copy)
                    if h % 2 == 0:
                        nc.vector.tensor_mul(
                            OP[h // 2][0:64, :], avps[h][0:64, :], bcs[0:64, :]
                        )
                    else:
                        ot = pr.tile([128, T], bf16, tag="otb", name="otb")
                        nc.vector.tensor_mul(
                            ot[0:64, :], avps[h][0:64, :], bcs[0:64, :]
                        )
                        nc.sync.dma_start(
                            out=OP[h // 2][64:128, :], in_=ot[0:64, :]
                        )

            def layernorm(src, gcol, bc_):
                mu = pp.tile([1, T], f32, tag="ps", name="ps")
                ms = pp.tile([1, T], f32, tag="ps", name="ps")
                for dc in range(8):
                    sq = tpr.tile([128, T], f32r, tag="sqr", name="sqr")
                    nc.scalar.square(sq[:], q32(src[dc][:]))
                    nc.tensor.matmul(
                        out=mu[:], lhsT=r(onesd_t[:]), rhs=r(src[dc][:]),
                        start=(dc == 0), stop=(dc == 7),
                    )
                    nc.tensor.matmul(
                        out=ms[:], lhsT=r(onesd_t[:]), rhs=r(sq[:]),
                        start=(dc == 0), stop=(dc == 7),
                    )
                mu_sb = sm.tile([1, T], f32r, tag="sm1", name="mu")
                nc.vector.tensor_copy(mu_sb[:], mu[:])
                t2 = sm.tile([1, T], f32, tag="sm1", name="t2")
                nc.vector.tensor_mul(t2[:], q32(mu_sb[:]), q32(mu_sb[:]))
                var = sm.tile([1, T], f32, tag="sm1", name="var")
                nc.vector.tensor_sub(var[:], ms[:], t2[:])
                nc.vector.tensor_scalar_add(var[:], var[:], EPS)
                std = sm.tile([1, T], f32, tag="sm1", name="std")
                nc.scalar.sqrt(std[:], var[:])
                rstd = sm.tile([1, T], f32r, tag="sm1", name="rstd")
                with nc.allow_low_precision(reason="f32r is fp32-width storage"):
                    nc.vector.reciprocal(rstd[:], std[:])
                mub = pp.tile([128, T], f32, tag="ps", name="ps")
                nc.tensor.matmul(
                    out=mub[:], lhsT=r(ones1[:]), rhs=r(mu_sb[:]), start=True, stop=True
                )
                rsb = pp.tile([128, T], f32, tag="ps", name="ps")
                nc.tensor.matmul(
                    out=rsb[:], lhsT=r(ones1[:]), rhs=r(rstd[:]), start=True, stop=True
                )
                rsb_sb = tps.tile([128, T], f32, tag="t512", name="t512")
                nc.vector.tensor_copy(rsb_sb[:], rsb[:])
                for dc in range(8):
                    t = tps.tile([128, T], f32, tag="t512", name="t512")
                    nc.vector.tensor_sub(t[:], q32(src[dc][:]), mub[:])
                    t2b = tps.tile([128, T], f32, tag="t512", name="t512")
                    nc.vector.tensor_mul(t2b[:], t[:], rsb_sb[:])
                    nc.scalar.activation(
                        XB[dc][:], t2b[:], AF.Identity,
                        bias=bcol(bc_ + dc), scale=bcol(gcol + dc),
                    )

            # ---------- Wo + residual + LN1 ----------
            for ec in range(8):
                pan = wpanel("wo", l, ec)
                ps = pp.tile([128, T], f32, tag="ps", name="ps")
                for dc in range(8):
                    nc.tensor.matmul(
                        out=ps[:], lhsT=pan[:, dc, :], rhs=OP[dc][:],
                        start=(dc == 0), stop=(dc == 7),
                    )
                nc.vector.scalar_tensor_tensor(
                    out=X2[ec][:], in0=ps[:], scalar=bcol(BO + ec), in1=XB[ec][:],
                    op0=ALU.add, op1=ALU.add,
                )
            layernorm(X2, L1G, L1B)

            # ---------- FFN: dff blocked, W2 partials accumulated in SBUF ----------
            for blk in range(8):
                fbt = []
                for k in range(4):
                    fc = blk * 4 + k
                    pan = wpanel("w1", l, fc)
                    ps = pp.tile([128, T], f32, tag="ps", name="ps")
                    for dc in range(8):
                        nc.tensor.matmul(
                            out=ps[:], lhsT=pan[:, dc, :], rhs=XB[dc][:],
                            start=(dc == 0), stop=(dc == 7),
                        )
                    fb = fbp.tile([128, T], bf16, tag="fblk", name="fblk")
                    fbt.append(fb)
                    nc.scalar.activation(
                        fb[:], ps[:], AF.Relu, bias=bcol(B1 + fc), scale=1.0
                    )
                for ec in range(8):
                    pan = wp.tile([128, 4, 128], bf16, tag="wpan", name="wpan")
                    nc.sync.dma_start(out=pan[:], in_=I["w2"][l, blk, ec])
                    ps = pp.tile([128, T], f32, tag="ps", name="ps")
                    for k in range(4):
                        nc.tensor.matmul(
                            out=ps[:], lhsT=pan[:, k, :], rhs=fbt[k][:],
                            start=(k == 0), stop=(k == 3),
                        )
                    if blk == 0:
                        nc.vector.scalar_tensor_tensor(
                            out=ACC[ec][:], in0=ps[:], scalar=bcol(B2 + ec),
                            in1=XB[ec][:], op0=ALU.add, op1=ALU.add,
                        )
                    else:
                        nc.vector.tensor_add(ACC[ec][:], q32(ACC[ec][:]), ps[:])
            layernorm(ACC, L2G, L2B)

        for ec in range(8):
            nc.sync.dma_start(out=y[ec * 128 : (ec + 1) * 128, :], in_=XB[ec][:])

    nc.compile()
    return nc


def _host_prep(inputs):
    import ml_dtypes

    bf16 = ml_dtypes.bfloat16
    g = {}
    Wqkv = np.asarray(inputs["Wqkv"], np.float32)
    bqkv = np.asarray(inputs["bqkv"], np.float32)
    sc = 1.0 / np.sqrt(HD)

    def panel8(wT):  # [L, 1024, 1024] -> [L, 8, 128, 8, 128]
        return np.ascontiguousarray(
            wT.reshape(L, 8, 128, 8, 128).transpose(0, 3, 2, 1, 4)
        ).astype(bf16)

    Wq = Wqkv[:, 0:D]  # [L, D, D] (out, in)
    Wk = Wqkv[:, D : 2 * D]
    Wv = Wqkv[:, 2 * D :]
    bv = bqkv[:, 2 * D :]
    wqT = np.ascontiguousarray(Wq.transpose(0, 2, 1)) * sc
    wkT = np.ascontiguousarray(Wk.transpose(0, 2, 1))
    g["wq"] = panel8(wqT)
    g["wk"] = panel8(wkT)
    g["wo"] = panel8(np.ascontiguousarray(np.asarray(inputs["Wo"], np.float32).transpose(0, 2, 1)))
    w1T = np.ascontiguousarray(np.asarray(inputs["W1"], np.float32).transpose(0, 2, 1))
    g["w1"] = np.ascontiguousarray(
        w1T.reshape(L, 8, 128, 32, 128).transpose(0, 3, 2, 1, 4)
    ).astype(bf16)
    w2T = np.ascontiguousarray(np.asarray(inputs["W2"], np.float32).transpose(0, 2, 1))
    g["w2"] = np.ascontiguousarray(
        w2T.reshape(L, 8, 4, 128, 8, 128).transpose(0, 1, 4, 3, 2, 5)
    ).astype(bf16)

    wv = np.zeros((L, D + 1, V_E), np.float32)
    for l in range(L):
        WvT = Wv[l].T
        for h in range(H):
            wv[l, :D, h * 65 : h * 65 + 64] = WvT[:, h * 64 : h * 64 + 64]
            wv[l, D, h * 65 : h * 65 + 64] = bv[l, h * 64 : h * 64 + 64]
    g["wv"] = wv.astype(bf16)

    ball = np.zeros((L, 128, 96), np.float32)
    ball[:, :, BQ : BQ + 8] = (bqkv[:, 0:D] * sc).reshape(L, 8, 128).transpose(0, 2, 1)
    ball[:, :, BK : BK + 8] = bqkv[:, D : 2 * D].reshape(L, 8, 128).transpose(0, 2, 1)
    ball[:, :, BO : BO + 8] = np.asarray(inputs["bo"], np.float32).reshape(L, 8, 128).transpose(0, 2, 1)
    ball[:, :, B2 : B2 + 8] = np.asarray(inputs["b2"], np.float32).reshape(L, 8, 128).transpose(0, 2, 1)
    ball[:, :, L1G : L1G + 8] = np.asarray(inputs["g1"], np.float32).reshape(L, 8, 128).transpose(0, 2, 1)
    ball[:, :, L1B : L1B + 8] = np.asarray(inputs["be1"], np.float32).reshape(L, 8, 128).transpose(0, 2, 1)
    ball[:, :, L2G : L2G + 8] = np.asarray(inputs["g2"], np.float32).reshape(L, 8, 128).transpose(0, 2, 1)
    ball[:, :, L2B : L2B + 8] = np.asarray(inputs["be2"], np.float32).reshape(L, 8, 128).transpose(0, 2, 1)
    ball[:, :, B1 : B1 + 32] = np.asarray(inputs["b1"], np.float32).reshape(L, 32, 128).transpose(0, 2, 1)
    g["ball"] = ball

    # boundary masks for chunks 4..8: keep iff c*128 + p <= q + 576
    selm = np.zeros((5, 128, 512), np.float32)
    p = np.arange(128)[:, None]
    q = np.arange(512)[None, :]
    for i, c in enumerate((4, 5, 6, 7, 8)):
        selm[i] = (c * 128 + p <= q + 576).astype(np.float32)
    g["selm"] = selm.astype(bf16)

    g["onesr"] = np.ones((1, 512), np.float32).astype(bf16)
    g["ones1"] = np.ones((1, 128), np.float32)
    g["onesd"] = np.full((128, 1), 1.0 / D, np.float32)

    xb = np.asarray(inputs["x"], np.float32).transpose(1, 0, 2)
    in_maps = []
    for c in range(8):
        b, hh = c // 2, c % 2
        pb = np.zeros((NC, 128, 1), np.float32)
        if hh == 0:
            pb[0:4] = NEG
            pb[8, 64:128] = NEG
        else:
            pb[8] = NEG
        m = dict(g)
        m["x0"] = np.ascontiguousarray(xb[b, hh * T : (hh + 1) * T, :].T).astype(bf16)
        m["pb"] = pb
        in_maps.append(m)
    return in_maps


def kernel(**inputs):
    from concourse.bass_utils import run_bass_kernel_spmd

    if "nc" not in _CACHE:
        _CACHE["nc"] = _build_program()
    nc = _CACHE["nc"]
    in_maps = _host_prep(inputs)
    res = run_bass_kernel_spmd(nc, in_maps, core_ids=list(range(8)))
    out = np.zeros((S, B, D), np.float32)
    for c in range(8):
        b, hh = c // 2, c % 2
        out[hh * T : (hh + 1) * T, b, :] = res.results[c]["y"].T.astype(np.float32)
    return out


# revision 26
# speedup vs baseline: 1.4278x; 1.1748x over previous
"""Trainium2 Bass kernel for nn_LocalTransformer (4-layer transformer,
d=1024, 16 heads, dff=4096, seq=1024, batch=4, causal + 64-lookahead mask).

Sharding: 8 cores = 4 samples x 2 sequence halves; each core owns 512
tokens. Attention context is a relative window of 1152 positions
(p = t - qoff + 512) so the mask predicate `p <= q + 576` is
core-independent; out-of-window positions are killed by per-core pad
biases added inside exp and boundary-chunk 0/1 mask multiplies.
K/V for remote positions move via a pairwise AllGather per layer; the
receiver computes peer = slot0 + slot1 - own (exact in fp32 for bf16).

Precision: all big matmuls in bf16 (weights host-converted, activations
shadowed in bf16) with fp32 PSUM accumulation; residual/LN in fp32.
bf16 weights enable fast-weight-load and LDW/MM overlap on the PE.
"""
import numpy as np

L, D, H, DFF, S, B = 4, 1024, 16, 4096, 1024, 4
HD = D // H  # 64
T = 512  # local tokens per core
WIN = 1152  # kv window positions (9 chunks of 128)
NC = 9
EPS = 1e-5
NEG = -30000.0
V_E = H * 65  # 1040: per head [V(64) | denominator-ones col]

_CACHE = {}

# packed per-layer bias/param columns in `ball` [L, 128, 96]
BQ, BK, BO, B2, L1G, L1B, L2G, L2B, B1 = 0, 8, 16, 24, 32, 40, 48, 56, 64


def _build_program():
    import concourse.bass as bass
    import concourse.tile as tile
    from concourse import bacc, mybir
    from contextlib import ExitStack

    f32, bf16, f32r = mybir.dt.float32, mybir.dt.bfloat16, mybir.dt.float32r
    AF = mybir.ActivationFunctionType
    ALU = mybir.AluOpType

    nc = bacc.Bacc("TRN2", target_bir_lowering=False, debug=False, num_devices=8)

    I = {}
    I["x0"] = nc.dram_tensor("x0", [D, T], bf16, kind="ExternalInput").ap()
    I["pb"] = nc.dram_tensor("pb", [NC, 128, 1], f32, kind="ExternalInput").ap()
    I["ball"] = nc.dram_tensor("ball", [L, 128, 96], f32, kind="ExternalInput").ap()
    for k, sh, dt in (
        ("wq", [L, 8, 128, 8, 128], bf16),
        ("wk", [L, 8, 128, 8, 128], bf16),
        ("wo", [L, 8, 128, 8, 128], bf16),
        ("w1", [L, 32, 128, 8, 128], bf16),
        ("w2", [L, 8, 8, 128, 4, 128], bf16),
        ("wv", [L, D + 1, V_E], bf16),
        ("selm", [5, 128, 512], bf16),
        ("onesr", [1, 512], bf16),
        ("ones1", [1, 128], f32r),
        ("onesd", [128, 1], f32r),
    ):
        I[k] = nc.dram_tensor(k, sh, dt, kind="ExternalInput").ap()
    y = nc.dram_tensor("y", [D, T], bf16, kind="ExternalOutput").ap()

    cck_in, cck_out, ccv_in, ccv_out = [], [], [], []
    for l in range(L):
        cck_in.append(nc.dram_tensor(f"ccki{l}", [D, 576], bf16, kind="Internal").ap())
        cck_out.append(nc.dram_tensor(f"ccko{l}", [2 * D, 576], bf16, kind="Internal").ap())
        ccv_in.append(nc.dram_tensor(f"ccvi{l}", [576, V_E], bf16, kind="Internal").ap())
        ccv_out.append(nc.dram_tensor(f"ccvo{l}", [1152, V_E], bf16, kind="Internal").ap())

    RG = [[0, 1], [2, 3], [4, 5], [6, 7]]
    SIDX = {4: 0, 5: 1, 6: 2, 7: 3, 8: 4}
    HGROUPS = [range(0, 6), range(6, 12), range(12, 16)]

    with tile.TileContext(nc) as tc, ExitStack() as ctx:
        pers = ctx.enter_context(tc.tile_pool(name="pers", bufs=1))
        XB = [pers.tile([128, T], bf16, tag=f"XB{i}", name=f"XB{i}") for i in range(8)]
        X2 = [pers.tile([128, T], f32r, tag=f"X2{i}", name=f"X2{i}") for i in range(8)]
        OP = [pers.tile([128, T], bf16, tag=f"OP{i}", name=f"OP{i}") for i in range(8)]
        Q = [pers.tile([128, T], bf16, tag=f"Q{i}", name=f"Qt{i}") for i in range(8)]
        KHo = [pers.tile([128, 512], bf16, tag=f"KHo{i}", name=f"KHo{i}") for i in range(8)]
        KHp = [pers.tile([128, 512], bf16, tag=f"KHp{i}", name=f"KHp{i}") for i in range(8)]
        KHn = [pers.tile([128, 128], bf16, tag=f"KHn{i}", name=f"KHn{i}") for i in range(8)]
        VT = [pers.tile([128, V_E], bf16, tag=f"VT{i}", name=f"VTt{i}") for i in range(NC)]
        ACC = [pers.tile([128, T], f32r, tag=f"ACC{i}", name=f"ACCt{i}") for i in range(8)]
        MT = [pers.tile([128, 512], bf16, tag=f"MT{i}", name=f"MTt{i}") for i in range(5)]
        ones_row = pers.tile([1, T], bf16, tag="ones_row", name="ones_row")
        ones1 = pers.tile([1, 128], f32r, tag="ones1", name="ones1t")
        onesd_t = pers.tile([128, 1], f32r, tag="onesd", name="onesdt")
        pb_t = [pers.tile([128, 1], f32, tag=f"pb{i}", name=f"pbt{i}") for i in range(NC)]
        D16 = pers.tile([16, T], f32, tag="D16", name="D16")
        R16 = pers.tile([16, T], f32, tag="R16", name="R16")

        wp = ctx.enter_context(tc.tile_pool(name="wp", bufs=6))  # weight panels
        smw = ctx.enter_context(tc.tile_pool(name="smw", bufs=3))  # small weights
        pp = ctx.enter_context(tc.tile_pool(name="pp", bufs=2, space="PSUM"))
        pav = ctx.enter_context(tc.tile_pool(name="pav", bufs=6, space="PSUM"))
        pr = ctx.enter_context(tc.tile_pool(name="pr", bufs=5))  # probs bf16
        tpv = ctx.enter_context(tc.tile_pool(name="tpv", bufs=2))  # [128,1040] f32
        tps = ctx.enter_context(tc.tile_pool(name="tps", bufs=3))  # [128,T] f32
        tpr = ctx.enter_context(tc.tile_pool(name="tpr", bufs=2))  # [128,T] f32r
        fx = ctx.enter_context(tc.tile_pool(name="fx", bufs=2))  # fixup bf16 loads
        sm = ctx.enter_context(tc.tile_pool(name="sm", bufs=3))  # [1,T] smalls
        fbp = ctx.enter_context(tc.tile_pool(name="fbp", bufs=8))  # FFN blocks
        rhp = ctx.enter_context(tc.tile_pool(name="rhp", bufs=2))  # [1,T] recip
        blp = ctx.enter_context(tc.tile_pool(name="blp", bufs=2))  # bias pack

        r = lambda ap: ap.bitcast(f32r)
        q32 = lambda ap: ap.bitcast(f32)

        nc.sync.dma_start(out=ones_row[:], in_=I["onesr"][:])
        nc.sync.dma_start(out=ones1[:], in_=I["ones1"][:])
        nc.sync.dma_start(out=onesd_t[:], in_=I["onesd"][:])
        for i in range(NC):
            nc.sync.dma_start(out=pb_t[i][:], in_=I["pb"][i])
        for i in range(5):
            nc.sync.dma_start(out=MT[i][:], in_=I["selm"][i])
        for i in range(8):
            nc.sync.dma_start(out=XB[i][:], in_=I["x0"][i * 128 : (i + 1) * 128, :])
        for i in range(8):
            nc.gpsimd.memset(KHn[i][:], 0.0)
        for i in range(NC):
            nc.gpsimd.memset(VT[i][:], 0.0)

        def wpanel(name, l, idx):
            pan = wp.tile([128, 8, 128], bf16, tag="wpan", name="wpan")
            nc.sync.dma_start(out=pan[:], in_=I[name][l, idx])
            return pan

        for l in range(L):
            ball = blp.tile([128, 96], f32, tag="ball", name="ball")
            nc.sync.dma_start(out=ball[:], in_=I["ball"][l])
            bcol = lambda j: ball[:, j : j + 1]

            # ---------- K projection -> KH[:, 512:1024], stage, AllGather ----------
            for ec in range(8):
                pan = wpanel("wk", l, ec)
                ps = pp.tile([128, T], f32, tag="ps", name="ps")
                for dc in range(8):
                    nc.tensor.matmul(
                        out=ps[:], lhsT=pan[:, dc, :], rhs=XB[dc][:],
                        start=(dc == 0), stop=(dc == 7),
                    )
                nc.scalar.activation(
                    KHo[ec][:], ps[:], AF.Identity, bias=bcol(BK + ec), scale=1.0
                )
                nc.sync.dma_start(
                    out=cck_in[l][ec * 128 : (ec + 1) * 128, 0:512],
                    in_=KHo[ec][:],
                )
                nc.sync.dma_start(
                    out=cck_in[l][ec * 128 : (ec + 1) * 128, 512:576],
                    in_=KHo[ec][:, 0:64],
                )
            nc.gpsimd.collective_compute(
                "AllGather", mybir.AluOpType.bypass, replica_groups=RG,
                ins=[cck_in[l][:]], outs=[cck_out[l][:]],
            )

            # ---------- V projection -> VT[4..7], stage, AllGather ----------
            for s0, s1 in ((0, 512), (512, 1024), (1024, V_E)):
                w = s1 - s0
                vpans = []
                for dc in range(8):
                    pan = smw.tile([128, 512], bf16, tag=f"vp{dc}", name=f"vp{dc}")
                    nc.sync.dma_start(
                        out=pan[:, 0:w],
                        in_=I["wv"][l, dc * 128 : (dc + 1) * 128, s0:s1],
                    )
                    vpans.append(pan)
                bia = smw.tile([1, 512], bf16, tag="vbias", name="vbias")
                nc.sync.dma_start(out=bia[:, 0:w], in_=I["wv"][l, D : D + 1, s0:s1])
                for tcx in range(4):
                    ps = pp.tile([128, T], f32, tag="ps", name="ps")
                    for dc in range(8):
                        nc.tensor.matmul(
                            out=ps[:, 0:w],
                            lhsT=XB[dc][:, tcx * 128 : (tcx + 1) * 128],
                            rhs=vpans[dc][:, 0:w],
                            start=(dc == 0), stop=False,
                        )
                    nc.tensor.matmul(
                        out=ps[:, 0:w],
                        lhsT=ones_row[:, 0:128],
                        rhs=bia[:, 0:w],
                        start=False, stop=True,
                    )
                    nc.scalar.activation(VT[4 + tcx][:, s0:s1], ps[:, 0:w], AF.Copy)
            for tcx in range(4):
                nc.sync.dma_start(
                    out=ccv_in[l][tcx * 128 : (tcx + 1) * 128, :], in_=VT[4 + tcx][:]
                )
            nc.sync.dma_start(out=ccv_in[l][512:576, :], in_=VT[4][0:64, :])
            nc.gpsimd.collective_compute(
                "AllGather", mybir.AluOpType.bypass, replica_groups=RG,
                ins=[ccv_in[l][:]], outs=[ccv_out[l][:]],
            )
            for pc in range(4, 8):  # local denominator ones-columns
                nc.gpsimd.memset(
                    VT[pc][:].rearrange("p (h c) -> p h c", c=65)[:, :, 64:65], 1.0
                )

            # ---------- Q projection ----------
            for ec in range(8):
                pan = wpanel("wq", l, ec)
                ps = pp.tile([128, T], f32, tag="ps", name="ps")
                for dc in range(8):
                    nc.tensor.matmul(
                        out=ps[:], lhsT=pan[:, dc, :], rhs=XB[dc][:],
                        start=(dc == 0), stop=(dc == 7),
                    )
                nc.scalar.activation(
                    Q[ec][:], ps[:], AF.Identity, bias=bcol(BQ + ec), scale=1.0
                )

            # ---------- remote KV fixup (consumes AllGather results) ----------
            for ec in range(8):
                ta = fx.tile([128, 576], bf16, tag="fxa", name="fxa")
                tb = fx.tile([128, 576], bf16, tag="fxb", name="fxb")
                nc.sync.dma_start(out=ta[:], in_=cck_out[l][ec * 128 : (ec + 1) * 128, :])
                nc.sync.dma_start(
                    out=tb[:], in_=cck_out[l][D + ec * 128 : D + (ec + 1) * 128, :]
                )
                ts = tpv.tile([128, 1040], f32, tag="fxv", name="fxv")
                nc.vector.tensor_add(ts[:, 0:576], ta[:], tb[:])
                nc.vector.tensor_sub(
                    KHp[ec][:], ts[:, 0:512], KHo[ec][:]
                )
                nc.vector.tensor_sub(
                    KHn[ec][:, 0:64], ts[:, 512:576], KHo[ec][:, 0:64]
                )
            for pc in range(4):
                ta = fx.tile([128, V_E], bf16, tag="fxa", name="fxa")
                tb = fx.tile([128, V_E], bf16, tag="fxb", name="fxb")
                nc.sync.dma_start(out=ta[:], in_=ccv_out[l][pc * 128 : (pc + 1) * 128, :])
                nc.sync.dma_start(
                    out=tb[:], in_=ccv_out[l][576 + pc * 128 : 576 + (pc + 1) * 128, :]
                )
                ts = tpv.tile([128, 1040], f32, tag="fxv", name="fxv")
                nc.vector.tensor_add(ts[:], ta[:], tb[:])
                nc.vector.tensor_sub(VT[pc][:], ts[:], VT[4 + pc][:])
            ta = fx.tile([128, V_E], bf16, tag="fxa", name="fxa")
            tb = fx.tile([128, V_E], bf16, tag="fxb", name="fxb")
            nc.sync.dma_start(out=ta[0:64, :], in_=ccv_out[l][512:576, :])
            nc.sync.dma_start(out=tb[0:64, :], in_=ccv_out[l][1088:1152, :])
            ts = tpv.tile([128, 1040], f32, tag="fxv", name="fxv")
            nc.vector.tensor_add(ts[0:64, :], ta[0:64, :], tb[0:64, :])
            nc.vector.tensor_sub(VT[8][0:64, :], ts[0:64, :], VT[4][0:64, :])
            for pc in (0, 1, 2, 3, 8):
                nc.gpsimd.memset(
                    VT[pc][:].rearrange("p (h c) -> p h c", c=65)[:, :, 64:65], 1.0
                )

            # ---------- attention, head groups with batched reciprocal ----------
            avps = {}
            for grp in HGROUPS:
                for h in grp:
                    par, kc = h % 2, h // 2
                    rows = slice(par * 64, par * 64 + 64)
                    av = pav.tile([128, T], f32, tag="av", name="av")
                    avps[h] = av
                    for idx, c in enumerate([4, 5, 6, 7, 0, 1, 2, 3, 8]):
                        if c < 4:
                            kt = KHp[kc][rows, c * 128 : (c + 1) * 128]
                        elif c < 8:
                            kt = KHo[kc][rows, (c - 4) * 128 : (c - 3) * 128]
                        else:
                            kt = KHn[kc][rows, 0:128]
                        sc = pp.tile([128, T], f32, tag="ps", name="ps")
                        nc.tensor.matmul(
                            out=sc[:],
                            lhsT=kt,
                            rhs=Q[kc][rows, :],
                            start=True, stop=True,
                        )
                        pt = pr.tile([128, T], bf16, tag="probs", name="probs")
                        nc.scalar.activation(
                            pt[:], sc[:], AF.Exp, bias=pb_t[c][:], scale=1.0
                        )
                        if c in SIDX:
                            nc.vector.tensor_mul(pt[:], pt[:], MT[SIDX[c]][:])
                        nc.tensor.matmul(
                            out=av[0:65, :],
                            lhsT=VT[c][:, h * 65 : h * 65 + 65],
                            rhs=pt[:],
                            start=(idx == 0), stop=(idx == 8),
                        )
                    dt_ = tps.tile([128, T], f32, tag="t512", name="t512")
                    nc.vector.tensor_copy(dt_[64:65, :], av[64:65, :])
                    j = h - grp.start
                    nc.sync.dma_start(out=D16[j : j + 1, :], in_=dt_[64:65, :])
                gs = grp.stop - grp.start
                nc.vector.reciprocal_approx_fast(R16[0:gs, :], D16[0:gs, :])
                for h in grp:
                    j = h - grp.start
                    rht = rhp.tile([1, T], f32r, tag="rh", name="rht")
                    nc.sync.dma_start(out=rht[:], in_=R16[j : j + 1, :].bitcast(f32r))
                    bc = pp.tile([128, T], f32, tag="ps", name="ps")
                    nc.tensor.matmul(
                        out=bc[0:64, :], lhsT=r(ones1[:, 0:64]), rhs=r(rht[:]),
                        start=True, stop=True,
                    )
                    bcs = tps.tile([128, T], f32, tag="t512", name="t512")
                    nc.vector.tensor_copy(bcs[0:64, :], bc[0:64, :])
                    if h % 2 == 0:
                        nc.vector.tensor_mul(
                            OP[h // 2][0:64, :], avps[h][0:64, :], bcs[0:64, :]
                        )
                    else:
                        ot = pr.tile([128, T], bf16, tag="otb", name="otb")
                        nc.vector.tensor_mul(
                            ot[0:64, :], avps[h][0:64, :], bcs[0:64, :]
                        )
                        nc.sync.dma_start(
                            out=OP[h // 2][64:128, :], in_=ot[0:64, :]
                        )

            def layernorm(src, gcol, bc_):
                mu = pp.tile([1, T], f32, tag="ps", name="ps")
                ms = pp.tile([1, T], f32, tag="ps", name="ps")
                for dc in range(8):
                    sq = tpr.tile([128, T], f32r, tag="sqr", name="sqr")
                    nc.scalar.square(sq[:], q32(src[dc][:]))
                    nc.tensor.matmul(
                        out=mu[:], lhsT=r(onesd_t[:]), rhs=r(src[dc][:]),
                        start=(dc == 0), stop=(dc == 7),
                    )
                    nc.tensor.matmul(
                        out=ms[:], lhsT=r(onesd_t[:]), rhs=r(sq[:]),
                        start=(dc == 0), stop=(dc == 7),
                    )
                mu_sb = sm.tile([1, T], f32r, tag="sm1", name="mu")
                nc.vector.tensor_copy(mu_sb[:], mu[:])
                t2 = sm.tile([1, T], f32, tag="sm1", name="t2")
                nc.vector.tensor_mul(t2[:], q32(mu_sb[:]), q32(mu_sb[:]))
                var = sm.tile([1, T], f32, tag="sm1", name="var")
                nc.vector.tensor_sub(var[:], ms[:], t2[:])
                nc.vector.tensor_scalar_add(var[:], var[:], EPS)
                vinv = sm.tile([1, T], f32, tag="sm1", name="vinv")
                nc.vector.reciprocal_approx_fast(vinv[:], var[:])
                rstd = sm.tile([1, T], f32r, tag="sm1", name="rstd")
                nc.scalar.sqrt(rstd[:], vinv[:])
                mub = pp.tile([128, T], f32, tag="ps", name="ps")
                nc.tensor.matmul(
                    out=mub[:], lhsT=r(ones1[:]), rhs=r(mu_sb[:]), start=True, stop=True
                )
                rsb = pp.tile([128, T], f32, tag="ps", name="ps")
                nc.tensor.matmul(
                    out=rsb[:], lhsT=r(ones1[:]), rhs=rstd[:], start=True, stop=True
                )
                rsb_sb = tps.tile([128, T], f32, tag="t512", name="t512")
                nc.vector.tensor_copy(rsb_sb[:], rsb[:])
                for dc in range(8):
                    t = tps.tile([128, T], f32, tag="t512", name="t512")
                    nc.vector.tensor_sub(t[:], q32(src[dc][:]), mub[:])
                    t2b = tps.tile([128, T], f32, tag="t512", name="t512")
                    nc.vector.tensor_mul(t2b[:], t[:], rsb_sb[:])
                    nc.scalar.activation(
                        XB[dc][:], t2b[:], AF.Identity,
                        bias=bcol(bc_ + dc), scale=bcol(gcol + dc),
                    )

            # ---------- Wo + residual + LN1 ----------
            for ec in range(8):
                pan = wpanel("wo", l, ec)
                ps = pp.tile([128, T], f32, tag="ps", name="ps")
                for dc in range(8):
                    nc.tensor.matmul(
                        out=ps[:], lhsT=pan[:, dc, :], rhs=OP[dc][:],
                        start=(dc == 0), stop=(dc == 7),
                    )
                nc.vector.scalar_tensor_tensor(
                    out=X2[ec][:], in0=ps[:], scalar=bcol(BO + ec), in1=XB[ec][:],
                    op0=ALU.add, op1=ALU.add,
                )
            layernorm(X2, L1G, L1B)

            # ---------- FFN: dff blocked, W2 partials accumulated in SBUF ----------
            for blk in range(8):
                fbt = []
                for k in range(4):
                    fc = blk * 4 + k
                    pan = wpanel("w1", l, fc)
                    ps = pp.tile([128, T], f32, tag="ps", name="ps")
                    for dc in range(8):
                        nc.tensor.matmul(
                            out=ps[:], lhsT=pan[:, dc, :], rhs=XB[dc][:],
                            start=(dc == 0), stop=(dc == 7),
                        )
                    fb = fbp.tile([128, T], bf16, tag="fblk", name="fblk")
                    fbt.append(fb)
                    nc.scalar.activation(
                        fb[:], ps[:], AF.Relu, bias=bcol(B1 + fc), scale=1.0
                    )
                for ec in range(8):
                    pan = wp.tile([128, 4, 128], bf16, tag="wpan", name="wpan")
                    nc.sync.dma_start(out=pan[:], in_=I["w2"][l, blk, ec])
                    ps = pp.tile([128, T], f32, tag="ps", name="ps")
                    for k in range(4):
                        nc.tensor.matmul(
                            out=ps[:], lhsT=pan[:, k, :], rhs=fbt[k][:],
                            start=(k == 0), stop=(k == 3),
                        )
                    if blk == 0:
                        nc.vector.scalar_tensor_tensor(
                            out=ACC[ec][:], in0=ps[:], scalar=bcol(B2 + ec),
                            in1=XB[ec][:], op0=ALU.add, op1=ALU.add,
                        )
                    else:
                        nc.vector.tensor_add(ACC[ec][:], q32(ACC[ec][:]), ps[:])
            layernorm(ACC, L2G, L2B)

        for ec in range(8):
            nc.sync.dma_start(out=y[ec * 128 : (ec + 1) * 128, :], in_=XB[ec][:])

    nc.compile()
    return nc


def _host_prep(inputs):
    import ml_dtypes

    bf16 = ml_dtypes.bfloat16
    g = {}
    Wqkv = np.asarray(inputs["Wqkv"], np.float32)
    bqkv = np.asarray(inputs["bqkv"], np.float32)
    sc = 1.0 / np.sqrt(HD)

    def panel8(wT):  # [L, 1024, 1024] -> [L, 8, 128, 8, 128]
        return np.ascontiguousarray(
            wT.reshape(L, 8, 128, 8, 128).transpose(0, 3, 2, 1, 4)
        ).astype(bf16)

    Wq = Wqkv[:, 0:D]  # [L, D, D] (out, in)
    Wk = Wqkv[:, D : 2 * D]
    Wv = Wqkv[:, 2 * D :]
    bv = bqkv[:, 2 * D :]
    wqT = np.ascontiguousarray(Wq.transpose(0, 2, 1)) * sc
    wkT = np.ascontiguousarray(Wk.transpose(0, 2, 1))
    g["wq"] = panel8(wqT)
    g["wk"] = panel8(wkT)
    g["wo"] = panel8(np.ascontiguousarray(np.asarray(inputs["Wo"], np.float32).transpose(0, 2, 1)))
    w1T = np.ascontiguousarray(np.asarray(inputs["W1"], np.float32).transpose(0, 2, 1))
    g["w1"] = np.ascontiguousarray(
        w1T.reshape(L, 8, 128, 32, 128).transpose(0, 3, 2, 1, 4)
    ).astype(bf16)
    w2T = np.ascontiguousarray(np.asarray(inputs["W2"], np.float32).transpose(0, 2, 1))
    g["w2"] = np.ascontiguousarray(
        w2T.reshape(L, 8, 4, 128, 8, 128).transpose(0, 1, 4, 3, 2, 5)
    ).astype(bf16)

    wv = np.zeros((L, D + 1, V_E), np.float32)
    for l in range(L):
        WvT = Wv[l].T
        for h in range(H):
            wv[l, :D, h * 65 : h * 65 + 64] = WvT[:, h * 64 : h * 64 + 64]
            wv[l, D, h * 65 : h * 65 + 64] = bv[l, h * 64 : h * 64 + 64]
    g["wv"] = wv.astype(bf16)

    ball = np.zeros((L, 128, 96), np.float32)
    ball[:, :, BQ : BQ + 8] = (bqkv[:, 0:D] * sc).reshape(L, 8, 128).transpose(0, 2, 1)
    ball[:, :, BK : BK + 8] = bqkv[:, D : 2 * D].reshape(L, 8, 128).transpose(0, 2, 1)
    ball[:, :, BO : BO + 8] = np.asarray(inputs["bo"], np.float32).reshape(L, 8, 128).transpose(0, 2, 1)
    ball[:, :, B2 : B2 + 8] = np.asarray(inputs["b2"], np.float32).reshape(L, 8, 128).transpose(0, 2, 1)
    ball[:, :, L1G : L1G + 8] = np.asarray(inputs["g1"], np.float32).reshape(L, 8, 128).transpose(0, 2, 1)
    ball[:, :, L1B : L1B + 8] = np.asarray(inputs["be1"], np.float32).reshape(L, 8, 128).transpose(0, 2, 1)
    ball[:, :, L2G : L2G + 8] = np.asarray(inputs["g2"], np.float32).reshape(L, 8, 128).transpose(0, 2, 1)
    ball[:, :, L2B : L2B + 8] = np.asarray(inputs["be2"], np.float32).reshape(L, 8, 128).transpose(0, 2, 1)
    ball[:, :, B1 : B1 + 32] = np.asarray(inputs["b1"], np.float32).reshape(L, 32, 128).transpose(0, 2, 1)
    g["ball"] = ball

    # boundary masks for chunks 4..8: keep iff c*128 + p <= q + 576
    selm = np.zeros((5, 128, 512), np.float32)
    p = np.arange(128)[:, None]
    q = np.arange(512)[None, :]
    for i, c in enumerate((4, 5, 6, 7, 8)):
        selm[i] = (c * 128 + p <= q + 576).astype(np.float32)
    g["selm"] = selm.astype(bf16)

    g["onesr"] = np.ones((1, 512), np.float32).astype(bf16)
    g["ones1"] = np.ones((1, 128), np.float32)
    g["onesd"] = np.full((128, 1), 1.0 / D, np.float32)

    xb = np.asarray(inputs["x"], np.float32).transpose(1, 0, 2)
    in_maps = []
    for c in range(8):
        b, hh = c // 2, c % 2
        pb = np.zeros((NC, 128, 1), np.float32)
        if hh == 0:
            pb[0:4] = NEG
            pb[8, 64:128] = NEG
        else:
            pb[8] = NEG
        m = dict(g)
        m["x0"] = np.ascontiguousarray(xb[b, hh * T : (hh + 1) * T, :].T).astype(bf16)
        m["pb"] = pb
        in_maps.append(m)
    return in_maps


def kernel(**inputs):
    from concourse.bass_utils import run_bass_kernel_spmd

    if "nc" not in _CACHE:
        _CACHE["nc"] = _build_program()
    nc = _CACHE["nc"]
    in_maps = _host_prep(inputs)
    res = run_bass_kernel_spmd(nc, in_maps, core_ids=list(range(8)))
    out = np.zeros((S, B, D), np.float32)
    for c in range(8):
        b, hh = c // 2, c % 2
        out[hh * T : (hh + 1) * T, b, :] = res.results[c]["y"].T.astype(np.float32)
    return out
